# revision 1
# baseline (speedup 1.0000x reference)
"""Trainium2 Bass kernel: per-head attention + residual + LayerNorm.

Problem shape: x [4, 2048, 1024], 16 heads of dk=64, causal softmax attention
with per-head Q/K/V linear projections, residual add, LayerNorm(D).

Sharding (8 cores): head-parallel. Core i owns heads (2i, 2i+1), i.e. feature
columns 128*i : 128*(i+1). Each core computes its feature slice of the output;
the only cross-core communication is a tiny per-batch AllReduce of LayerNorm
partial sums (sum and sum-of-squares over each core's 128 features). The host
shards/gathers and pre-transposes x (the [dk, S] layout each head needs).

Per-core kernel design (bf16 matmuls, fp32 PSUM accumulation):
- Scores via a host-precomputed bilinear form: scores^T = xh_aug^T @ z with
  z = [M @ xh^T + u | beta-row], M = Wk (Wq/sqrt(dk))^T. One projected tensor
  (z) instead of Q and K halves the PSUM->SBUF copies; x^T itself (with a
  built-in ones row for the bias terms) is the stationary matmul operand.
- Flash-style t-outer loop over 1024-col query halves; scores accumulate in
  PSUM [128,1024] chunks on a dedicated 3-deep ring (6 banks) that serves
  ONLY the score->exp pipeline; exp runs on ScalarE straight from PSUM in
  one instruction per (t-block, half), P is bf16 in SBUF. All other PSUM
  users (split O^T accumulators, projection staging, epilogue transposes)
  live on a separate ring of fast-release 1-bank [*,512] slots (2 banks),
  so no phase ever blocks the score pipeline's buffers.
- Causal mask: an identity-matmul accumulates -40 onto the upper triangle of
  the diagonal 128-block before exp (no vector-engine masking); sub-diagonal
  dead zones are simply never read by PV.
- PV accumulates O^T [65,1024] in PSUM with a ones-augmented V, so softmax
  denominators ride along as row 64; per 128-tile PE-transposes then let a
  single fused DVE op do (O*1/l + x) with the row-sum accumulated for free.
- rstd = exp(-0.5*ln(var+eps)) keeps every activation in one ACT table set
  (natural_log_exp_and_others; enforced by filtering the set map at compile).
- Emission is software-pipelined: the next pair's z/V projection is emitted
  mid-way through the current score loop, the transpose/normalize epilogue is
  deferred one unit, and per-half LayerNorm stats AllReduce in 8 small chunks
  so TensorE/ScalarE/VectorE and the collective overlap across units.

Self-contained: hardcodes all shapes; no sibling imports.
"""

import os
import numpy as np
import ml_dtypes

import concourse.bass as bass
import concourse.bacc as bacc
import concourse.mybir as mybir
from concourse.tile import TileContext
from concourse.bass_utils import run_bass_kernel_spmd

B, S, D, H = 4, 2048, 1024, 16
NCORES = 8
HPC = H // NCORES          # heads per core = 2
DK = D // H                # 64
DC = HPC * DK              # 128 feature cols per core
NT = S // 128              # 16 row tiles of 128
EPS = 1e-5
MASKNEG = -40.0
SPBUFS = 3
OPBUFS = 1
EPI_LAG = 1
HOOKJ = 6
QKBUFS = 3
PBUFS = 6
BF = mybir.dt.bfloat16
F32 = mybir.dt.float32
BF_NP = ml_dtypes.bfloat16
RG = [list(range(NCORES))]
A = mybir.AluOpType
AF = mybir.ActivationFunctionType

LAST_RESULTS = None  # BassKernelResults of the last run (for test harness)


def _build_graph(apply_affine: bool, B_: int = B, S_: int = S, rg=None, fake_ar: bool = False) -> bass.Bass:
    nc = bacc.Bacc()
    NT_ = S_ // 128
    if rg is None:
        rg = RG

    xt = nc.declare_dram_parameter("xt", [B_, HPC, DK + 1, S_], BF, isOutput=False)
    xs = nc.declare_dram_parameter("xs", [B_, S_, DC], F32, isOutput=False)
    wpack = nc.declare_dram_parameter(
        "wpack", [DK, HPC * (DK + 1) + HPC * DK], BF, isOutput=False
    )
    zb = nc.declare_dram_parameter("zb", [DK + 1, HPC], F32, isOutput=False)
    bv16 = nc.declare_dram_parameter("bv16", [HPC, 128, 16 * DK], F32, isOutput=False)
    if apply_affine:
        gam = nc.declare_dram_parameter("gam", [128, DC], F32, isOutput=False)
        bet = nc.declare_dram_parameter("bet", [128, DC], F32, isOutput=False)
    out = nc.declare_dram_parameter("out", [B_, S_, DC], F32, isOutput=True)

    # constants baked into the NEFF
    idn_h = nc.inline_tensor(np.eye(DK + 1, dtype=np.float32), name="idn")
    trineg_np = np.where(
        np.arange(128)[:, None] > np.arange(128)[None, :], MASKNEG, 0.0
    ).astype(np.float32)
    imask_h = nc.inline_tensor(
        np.concatenate([np.eye(128, dtype=np.float32), trineg_np], axis=1).astype(
            BF_NP
        ),
        name="imask",
    )

    # collective bounce buffers: LayerNorm stats per (batch, s-half):
    # [2(sum,sumsq), 128 rows, tiles-in-half]
    NHALF = (S_ + 1023) // 1024
    NTH = NT_ // NHALF
    stats_in = nc.dram_tensor("stats_in", [B_, NHALF, 2, 128, NTH], F32)
    stats_out = nc.dram_tensor(
        "stats_out", [B_, NHALF, 2, 128, NTH], F32, addr_space="Shared"
    )

    with TileContext(nc) as tc:
        with (
            tc.tile_pool(name="consts", bufs=1) as cpool,
            tc.tile_pool(name="sb", bufs=2) as sb,
            tc.tile_pool(name="ps", bufs=1, space="PSUM") as ps,
        ):
            # ---- load constants (single coalesced DMAs, first-needed first) ----
            wp_t = cpool.tile([DK, HPC * (DK + 1) + HPC * DK], BF, tag="wp")
            nc.sync.dma_start(out=wp_t[:], in_=wpack[:, :])
            zw_t = wp_t[:][:, 0 : HPC * (DK + 1)]
            wv_t = wp_t[:][:, HPC * (DK + 1) : HPC * (DK + 1) + HPC * DK]
            # queue order tuned for the first exp: wpack, then the first
            # half of head-0 x^T (all the first projection needs), then the
            # small constants the first z-copy and diagonal mask need, then
            # the rest of batch-0 x^T
            xth0 = [
                sb.tile([DK + 1, S_], BF, tag="xth", name=f"xth0_{h2}", bufs=B_ * HPC)
                for h2 in range(HPC)
            ]
            nc.sync.dma_start(out=xth0[0][:, 0 : S_ // 2], in_=xt[0, 0, :, 0 : S_ // 2])
            zbq_t = cpool.tile([DK + 1, HPC], F32, tag="zb")
            nc.sync.dma_start(out=zbq_t[:], in_=zb[:, :])
            imaskq_t = cpool.tile([128, 256], BF, tag="imask")
            nc.sync.dma_start(out=imaskq_t[:], in_=imask_h[:, :])
            nc.sync.dma_start(out=xth0[0][:, S_ // 2 : S_], in_=xt[0, 0, :, S_ // 2 : S_])
            nc.sync.dma_start(out=xth0[1][:, 0 : S_ // 2], in_=xt[0, 1, :, 0 : S_ // 2])
            nc.sync.dma_start(out=xth0[1][:, S_ // 2 : S_], in_=xt[0, 1, :, S_ // 2 : S_])
            idn_t = cpool.tile([DK + 1, DK + 1], F32, tag="idn")
            nc.gpsimd.dma_start(out=idn_t[:], in_=idn_h[:, :])
            bv16_t = cpool.tile([128, HPC * 16 * DK], F32, tag="bv16")
            for h in range(HPC):
                nc.gpsimd.dma_start(
                    out=bv16_t[:, 16 * DK * h : 16 * DK * (h + 1)], in_=bv16[h]
                )
            if apply_affine:
                gam_t = cpool.tile([128, DC], F32, tag="gam")
                nc.sync.dma_start(out=gam_t[:], in_=gam[:, :])
                bet_t = cpool.tile([128, DC], F32, tag="bet")
                nc.sync.dma_start(out=bet_t[:], in_=bet[:, :])

            zb_t = zbq_t
            idn128_t = imaskq_t[:][:, 0:128]
            maskt_t = imaskq_t[:][:, 128:256]
            eps_t = cpool.tile([128, 1], F32, tag="eps")
            nc.vector.memset(eps_t[:], EPS)

            pending_epi = [None]

            def _emit_stats(b, y_b, acc, hs, he, ch):
                # LayerNorm partial stats + AllReduce for one s-half
                t0, t1 = hs // 128, he // 128
                nth = t1 - t0
                sums = sb.tile([128, NTH], F32, tag="sums", bufs=3)
                nc.vector.tensor_add(
                    sums[:, 0:nth], acc[0][:, t0:t1], acc[1][:, t0:t1]
                )
                sq = sb.tile([128, NTH], F32, tag="sq", bufs=3)
                for i in range(t0, t1):
                    scr = sb.tile([128, 128], F32, tag="scr")
                    nc.vector.scalar_tensor_tensor(
                        scr[:],
                        y_b[:, 128 * i : 128 * i + 128],
                        1.0,
                        y_b[:, 128 * i : 128 * i + 128],
                        A.mult,
                        A.mult,
                        accum_out=sq[:, i - t0 : i - t0 + 1],
                    )
                nc.sync.dma_start(out=stats_in[b, ch, 0], in_=sums[:, 0:nth])
                nc.sync.dma_start(out=stats_in[b, ch, 1], in_=sq[:, 0:nth])
                if fake_ar:
                    nc.sync.dma_start(out=stats_out[b, ch], in_=stats_in[b, ch])
                else:
                    nc.gpsimd.collective_compute(
                        "AllReduce",
                        A.add,
                        replica_groups=rg,
                        ins=[stats_in[b, ch].opt()],
                        outs=[stats_out[b, ch].opt()],
                    )

            def emit_ln(b, ch, y_b):
                t0 = ch * NTH
                red = sb.tile([128, 2 * NTH], F32, tag="red", bufs=3)
                nc.sync.dma_start(
                    out=red[:].rearrange("p (c t) -> p c t", t=NTH),
                    in_=stats_out[b, ch].rearrange("c p t -> p c t"),
                )
                mean = sb.tile([128, NTH], F32, tag="mean", bufs=3)
                nc.vector.tensor_scalar(
                    mean[:], red[:, 0:NTH], 1.0 / D, None, A.mult
                )
                msq = sb.tile([128, NTH], F32, tag="msq", bufs=3)
                nc.vector.tensor_mul(msq[:], mean[:], mean[:])
                var = sb.tile([128, NTH], F32, tag="var", bufs=3)
                nc.vector.scalar_tensor_tensor(
                    var[:], red[:, NTH : 2 * NTH], 1.0 / D, msq[:], A.mult,
                    A.subtract,
                )
                lnv = sb.tile([128, NTH], F32, tag="lnv", bufs=3)
                nc.scalar.activation(lnv[:], var[:], AF.Ln, bias=eps_t[:])
                rstd = sb.tile([128, NTH], F32, tag="rstd", bufs=3)
                nc.scalar.activation(rstd[:], lnv[:], AF.Exp, scale=-0.5)
                ostb = sb.tile([128, 128 * NTH], F32, tag="ost", bufs=2)
                for k in range(NTH):
                    i = t0 + k
                    nc.vector.tensor_scalar(
                        ostb[:, 128 * k : 128 * k + 128],
                        y_b[:, 128 * i : 128 * i + 128],
                        mean[:, k : k + 1],
                        rstd[:, k : k + 1],
                        A.subtract,
                        A.mult,
                    )
                    if apply_affine:
                        nc.vector.tensor_mul(
                            ostb[:, 128 * k : 128 * k + 128],
                            ostb[:, 128 * k : 128 * k + 128],
                            gam_t[:],
                        )
                        nc.vector.tensor_add(
                            ostb[:, 128 * k : 128 * k + 128],
                            ostb[:, 128 * k : 128 * k + 128],
                            bet_t[:],
                        )
                eng = nc.gpsimd if ((b * NHALF + ch) % 2 == 0 and b < B_ - 1) else nc.sync
                eng.dma_start(
                    out=out[b, 128 * t0 : 128 * (t0 + NTH), :].rearrange(
                        "(i p) d -> p i d", p=128
                    ),
                    in_=ostb[:].rearrange("p (i d) -> p i d", d=128),
                )

            y_tiles = {}
            bstate = {}
            pstate = {}
            pw = min(1024, S_)
            NP = B_ * HPC

            def emit_proj(pair):
                b, hh = divmod(pair, HPC)
                if hh == 0:
                    if b == 0:
                        xth = xth0
                    else:
                        xth = [None, None]
                        for h2 in range(HPC):
                            xth[h2] = sb.tile(
                                [DK + 1, S_], BF, tag="xth", name=f"xth{b}_{h2}", bufs=B_ * HPC
                            )
                            nc.sync.dma_start(
                                out=xth[h2][:, 0 : S_ // 2], in_=xt[b, h2, :, 0 : S_ // 2]
                            )
                            nc.sync.dma_start(
                                out=xth[h2][:, S_ // 2 : S_], in_=xt[b, h2, :, S_ // 2 : S_]
                            )
                    xs_b = sb.tile([128, S_], F32, tag="xs", name=f"xs{b}")
                    y_b = sb.tile([128, S_], F32, tag=f"y{b}", name=f"y{b}")
                    y_tiles[b] = y_b
                    bstate[b] = (xth, xs_b, y_b, {})
                    need_xs_dma = True
                else:
                    need_xs_dma = False
                xth, xs_b, y_b, accs = bstate[b]
                xh = xth[hh]
                # z = [M @ xh^T + u | beta-row]: scores become xh_aug^T @ z
                z = sb.tile([DK + 1, S_], BF, tag="z", name=f"z{pair}", bufs=NP)
                for c in range(S_ // 512):
                    zp = ps.tile([128, 512], F32, tag="op", bufs=2, name=f"zp{c}")
                    nc.tensor.matmul(
                        zp[0 : DK + 1, :],
                        lhsT=zw_t[:, (DK + 1) * hh : (DK + 1) * (hh + 1)],
                        rhs=xh[0:DK, 512 * c : 512 * c + 512],
                        start=True,
                        stop=True,
                    )
                    nc.vector.tensor_scalar(
                        z[:, 512 * c : 512 * c + 512],
                        zp[0 : DK + 1, :],
                        zb_t[:, hh : hh + 1],
                        None,
                        A.add,
                    )
                # V with bias, ones-augmented: v = [V | 1] blocks of 65 cols
                v = sb.tile([128, NT_ * (DK + 1)], BF, tag="v", name=f"v{pair}", bufs=NP)
                v3 = v[:].rearrange("p (t w) -> p t w", w=DK + 1)
                nc.vector.memset(v3[:, :, DK : DK + 1], 1.0)
                gv = min(8, NT_)
                for g in range(NT_ // gv):
                    vp = ps.tile([128, 512], F32, tag="op", bufs=2, name=f"vp{g}")
                    for u in range(gv):
                        j = gv * g + u
                        nc.tensor.matmul(
                            vp[:, DK * u : DK * u + DK],
                            lhsT=xh[0:DK, 128 * j : 128 * j + 128],
                            rhs=wv_t[:, hh * DK : hh * DK + DK],
                            start=True,
                            stop=True,
                        )
                    nc.vector.tensor_tensor(
                        v3[:, gv * g : gv * g + gv, 0:DK],
                        vp[:, 0 : gv * DK].rearrange("q (t w) -> q t w", w=DK),
                        bv16_t[:].rearrange("q (h t w) -> q (h t) w", h=HPC, w=DK)[
                            :, hh * 16 : hh * 16 + gv, :
                        ],
                        A.add,
                    )
                if need_xs_dma:
                    nc.sync.dma_start(
                        out=xs_b[:].rearrange("p (i d) -> p i d", d=128),
                        in_=xs[b].rearrange("(i p) d -> p i d", p=128),
                    )
                acc_h = sb.tile([128, NT_], F32, tag=f"acc{hh}", name=f"acc{pair}", bufs=B_)
                accs[hh] = acc_h
                pstate[pair] = (xh, z, v3, acc_h)

            def emit_jhalf(pair, hs, mid_hook=None):
                """Score/exp/PV loop for one 1024-col s-half; returns the
                deferred transpose/normalize epilogue closure."""
                b, hh = divmod(pair, HPC)
                xh, z, v3, acc_h = pstate[pair]
                _, xs_b, y_b, accs = bstate[b]
                he = min(S_, hs + 1024)
                w = he - hs
                opA = ps.tile([DK + 1, 512], F32, tag="op", bufs=2)
                opB = ps.tile([DK + 1, 512], F32, tag="op", bufs=2)
                prev_pv = None
                for j in range(he // 128):
                    s0 = 128 * j
                    rel = s0 - hs
                    p = sb.tile([128, 1024], BF, tag="p", bufs=PBUFS)
                    sp = ps.tile([128, 1024], F32, tag="sp", bufs=SPBUFS)
                    if rel < 0:
                        ss = 0
                        while ss < w:
                            sl = min(512, w - ss)
                            nc.tensor.matmul(
                                sp[:, ss : ss + sl],
                                lhsT=xh[:, s0 : s0 + 128],
                                rhs=z[:, hs + ss : hs + ss + sl],
                                start=True,
                                stop=True,
                            )
                            ss += sl
                        lo = 0
                    else:
                        lo = rel
                        nc.tensor.matmul(
                            sp[:, rel : rel + 128],
                            lhsT=idn128_t,
                            rhs=maskt_t,
                            start=True,
                            stop=False,
                            skip_group_check=True,
                        )
                        nc.tensor.matmul(
                            sp[:, rel : rel + 128],
                            lhsT=xh[:, s0 : s0 + 128],
                            rhs=z[:, s0 : s0 + 128],
                            start=False,
                            stop=True,
                            skip_group_check=True,
                        )
                        ss = rel + 128
                        while ss < w:
                            sl = min(512 - (ss % 512), w - ss)
                            nc.tensor.matmul(
                                sp[:, ss : ss + sl],
                                lhsT=xh[:, s0 : s0 + 128],
                                rhs=z[:, hs + ss : hs + ss + sl],
                                start=True,
                                stop=True,
                            )
                            ss += sl
                    nc.scalar.activation(p[:, lo:w], sp[:, lo:w], AF.Exp)

                    # PV deferred by one j so PE computes S_{j+1} while the
                    # ACT engine exps j (avoids PE stalling on exp latency)
                    def _pv(j=j, p=p, lo=lo):
                        cs = lo
                        while cs < w:
                            ce = min(512 * (cs // 512) + 512, w)
                            gc = (hs + cs) // 512
                            opt = opA if cs < 512 else opB
                            nc.tensor.matmul(
                                opt[:, cs % 512 : cs % 512 + (ce - cs)],
                                lhsT=v3[:, j, :],
                                rhs=p[:, cs:ce],
                                start=(j == 0),
                                stop=(j == min(he // 128 - 1, 4 * gc + 3)),
                                skip_group_check=True,
                            )
                            cs = ce

                    if prev_pv is not None:
                        prev_pv()
                    prev_pv = _pv
                    if j == min(HOOKJ, he // 128 - 1) and mid_hook is not None:
                        mid_hook()
                if prev_pv is not None:
                    prev_pv()
                # drain O^T; transpose/normalize deferred
                ot = sb.tile([DK + 1, 1024], F32, tag="ot", bufs=2 + EPI_LAG)
                nc.vector.tensor_copy(ot[:, 0 : min(512, w)], opA[:, 0 : min(512, w)])
                if w > 512:
                    nc.vector.tensor_copy(ot[:, 512:w], opB[:, 0 : w - 512])

                def _epilogue():
                    nk = he // 128 - hs // 128
                    # transposes staged in two 1-bank tiles on the op ring
                    # (freed by the early accumulator drains), keeping the
                    # score ring untouched by the epilogue
                    tps = [
                        ps.tile([128, 512], F32, tag="op", bufs=2, name=f"tp{g}")
                        for g in range((nk + 3) // 4)
                    ]
                    for i in range(hs // 128, he // 128):
                        k = i - hs // 128
                        tp = tps[k // 4]
                        nc.tensor.transpose(
                            tp[:, 128 * (k % 4) : 128 * (k % 4) + DK + 1],
                            ot[:, 128 * i - hs : 128 * i - hs + 128],
                            idn_t[:],
                        )
                    r8 = sb.tile([128, 8], F32, tag="r8", bufs=3)
                    for g, tp in enumerate(tps):
                        gn = min(4, nk - 4 * g)
                        nc.vector.reciprocal(
                            r8[:, 4 * g : 4 * g + gn],
                            tp[:].rearrange("q (k c) -> q k c", c=128)[
                                :, 0:gn, DK : DK + 1
                            ],
                        )
                    for i in range(hs // 128, he // 128):
                        k = i - hs // 128
                        tp = tps[k // 4]
                        nc.vector.scalar_tensor_tensor(
                            y_b[:, 128 * i + DK * hh : 128 * i + DK * hh + DK],
                            tp[:, 128 * (k % 4) : 128 * (k % 4) + DK],
                            r8[:, k : k + 1],
                            xs_b[:, 128 * i + DK * hh : 128 * i + DK * hh + DK],
                            A.mult,
                            A.add,
                            accum_out=acc_h[:, i : i + 1],
                        )
                    if hh == HPC - 1:
                        _emit_stats(b, y_b, accs, hs, he, hs // 1024)

                return _epilogue

            emit_proj(0)
            pending = []
            for pair in range(NP):
                for k, hs in enumerate(range(0, S_, 1024)):
                    hook = None
                    if k == 0 and pair + 1 < NP:
                        hook = (lambda pr=pair: emit_proj(pr + 1))
                    epi = emit_jhalf(pair, hs, mid_hook=hook)
                    pending.append(epi)
                    if len(pending) > EPI_LAG:
                        pending.pop(0)()
            for e in pending:
                e()

            for b in range(B_):
                for ch in range(NHALF):
                    emit_ln(b, ch, y_tiles[b])


    # Restrict Exp/Ln to the shared natural_log_exp_and_others table set so
    # the whole kernel uses one ACT table load (indices preserved).
    import concourse.bacc as _bacc_mod

    _orig_tables = _bacc_mod.get_activation_tables

    def _filtered_tables(arch):
        out = {}
        for name, fns in _orig_tables(arch).items():
            if name != "natural_log_exp_and_others":
                fns = set(fns) - {AF.Exp, AF.Ln}
            out[name] = fns
        return out

    _bacc_mod.get_activation_tables = _filtered_tables
    try:
        nc.compile()
    finally:
        _bacc_mod.get_activation_tables = _orig_tables
    return nc


_GRAPH_CACHE = {}


def _get_graph(apply_affine: bool) -> bass.Bass:
    if apply_affine not in _GRAPH_CACHE:
        _GRAPH_CACHE[apply_affine] = _build_graph(apply_affine)
    return _GRAPH_CACHE[apply_affine]


def _prep_in_maps(x, Wq, bq, Wk, bk, Wv, bv, gamma, beta, apply_affine):
    scale = 1.0 / np.sqrt(np.float32(DK))
    in_maps = []
    for i in range(NCORES):
        dsl = slice(DC * i, DC * (i + 1))
        hsl = slice(HPC * i, HPC * (i + 1))
        x_sl = x[:, :, dsl]
        xt_full = x_sl.transpose(0, 2, 1).reshape(x.shape[0], HPC, DK, x.shape[1])
        xt_aug = np.concatenate(
            [xt_full, np.ones((x.shape[0], HPC, 1, x.shape[1]), np.float32)], axis=2
        )
        Wq_s = (Wq[hsl] * scale).astype(np.float64)
        bq_s = (bq[hsl] * scale).astype(np.float64)
        Wk_h = Wk[hsl].astype(np.float64)
        bk_h = bk[hsl].astype(np.float64)
        M = np.einsum("hde,hfe->hdf", Wk_h, Wq_s)      # [h, dK, dQ]
        u = np.einsum("hde,he->hd", Wk_h, bq_s)        # alpha coeffs (per t)
        wvec = np.einsum("hde,he->hd", Wq_s, bk_h)     # beta coeffs (per s)
        cconst = np.einsum("he,he->h", bk_h, bq_s)
        # lhsT for z: [d', dK | wvec]; z rows 0..63 = M@xh^T + u, row 64 = xh.w + c
        zw_np = np.concatenate(
            [M.transpose(0, 2, 1), wvec[:, :, None]], axis=2
        )  # [h, d'(=dQ... contraction dim), dK+1]
        zb_np = np.concatenate([u, cconst[:, None]], axis=1)[:, :, None]
        m = {
            "xt": np.ascontiguousarray(xt_aug).astype(BF_NP),
            "xs": np.ascontiguousarray(x_sl),
            "wpack": np.ascontiguousarray(
                np.concatenate(
                    [zw_np[0], zw_np[1], Wv[hsl][0], Wv[hsl][1]], axis=1
                )
            ).astype(BF_NP),
            "zb": np.ascontiguousarray(zb_np[:, :, 0].T).astype(np.float32),
            "bv16": np.ascontiguousarray(
                np.tile(bv[hsl][:, None, :], (1, 128, 16))
            ).astype(np.float32),
        }
        if apply_affine:
            m["gam"] = np.ascontiguousarray(
                np.tile(gamma[dsl][None, :], (128, 1))
            ).astype(np.float32)
            m["bet"] = np.ascontiguousarray(
                np.tile(beta[dsl][None, :], (128, 1))
            ).astype(np.float32)
        in_maps.append(m)
    return in_maps


def kernel(x, Wq, bq, Wk, bk, Wv, bv, gamma, beta):
    global LAST_RESULTS
    x = np.asarray(x, np.float32)
    Wq = np.asarray(Wq, np.float32)
    bq = np.asarray(bq, np.float32)
    Wk = np.asarray(Wk, np.float32)
    bk = np.asarray(bk, np.float32)
    Wv = np.asarray(Wv, np.float32)
    bv = np.asarray(bv, np.float32)
    gamma = np.asarray(gamma, np.float32)
    beta = np.asarray(beta, np.float32)

    apply_affine = not (
        np.allclose(gamma, 1.0, atol=0.0, rtol=0.0)
        and np.allclose(beta, 0.0, atol=0.0, rtol=0.0)
    )
    fake_ar = bool(int(os.environ.get("KERNEL_FAKE_AR", "0")))
    nc = _get_graph(apply_affine) if not fake_ar else _build_graph(apply_affine, fake_ar=True)

    in_maps = _prep_in_maps(x, Wq, bq, Wk, bk, Wv, bv, gamma, beta, apply_affine)

    res = run_bass_kernel_spmd(
        nc,
        in_maps,
        core_ids=list(range(NCORES)),
        trace=bool(int(os.environ.get("KERNEL_TRACE", "0"))),
    )
    LAST_RESULTS = res
    outs = [np.asarray(r["out"], np.float32) for r in res.results]
    return np.concatenate(outs, axis=2)


if __name__ == "__main__":
    nc = _build_graph(False)
    print("graph built ok:", len(nc.inst_map), "instructions")



# revision 19
# speedup vs baseline: 1.0369x; 1.0369x over previous
"""Trainium2 Bass kernel: per-head attention + residual + LayerNorm.

Problem shape: x [4, 2048, 1024], 16 heads of dk=64, causal softmax attention
with per-head Q/K/V linear projections, residual add, LayerNorm(D).

Sharding (8 cores): head-parallel. Core i owns heads (2i, 2i+1) = feature
columns 128*i : 128*(i+1). Only cross-core traffic: per-(batch,unit) AllReduce
of LayerNorm partial sums.

v2 design (vs the 202us baseline, whose bottleneck was the ACT engine at 77%
busy running exp over the causal score area):
- Softmax-invariance scores: sp[t,s] = x_t^T (Wk Wq^T/sqrt(dk)) x_s + u.x_t
  (query-only bias terms cancel in softmax). The per-key bias u.x_t is
  computed on the HOST and folded into the exp instruction's per-partition
  bias operand - no bias-row augmentation, no on-device bias adds.
- zz = block-diag(A_h0^T, A_h1^T) @ xx projects BOTH heads in one matmul
  stream ([128,S]); per-head scores contract over a 64-partition slice.
- exp is split across the ACT engine (true Exp) and DVE (Schraudolph:
  bf16 = bitcast(int16(184.665*s + bias)), a single tensor_scalar with
  ~1.8% sigma error that mostly cancels in softmax). A build-time greedy
  balancer also assigns the zz/v PSUM->SBUF copies to ACT (AF.Copy, same
  act table) or DVE. GPSIMD cannot touch PSUM, so Pool instead absorbs the
  SBUF-only LayerNorm/stats elementwise work.
- PV reoriented to P^T V: out[s-chunk 128, 65] with lhsT = P chunk; free
  size 65 per matmul (vs 512) halves PE PV time, writes y directly (no PE
  transposes, no O^T drain copies), and the rhs ones-column lands the
  softmax denominator in chunk col 64. V bias is pre-added into xs on host.
- Query-units of (896, 896, 256) cols keep the PV chunk tile (nch x 65 fp32
  <= 455) inside one PSUM bank; processed [u1, u2, u0] so the last
  (batch,unit) stats AllReduce covers the smallest unit (short tail).
- LayerNorm output (emit_ln) is emitted ~one pair after its AllReduce
  fires, spreading Pool/out-DMA work over compute instead of a serial tail.

Self-contained: hardcodes all shapes; no sibling imports.
"""

import os
import numpy as np
import ml_dtypes

import concourse.bass as bass
import concourse.bacc as bacc
import concourse.mybir as mybir
from concourse.tile import TileContext
from concourse.bass_utils import run_bass_kernel_spmd

B, S, D, H = 4, 2048, 1024, 16
NCORES = 8
HPC = H // NCORES          # heads per core = 2
DK = D // H                # 64
DC = HPC * DK              # 128 feature cols per core
NT = S // 128              # 16 row tiles of 128
EPS = 1e-5
MASKNEG = -40.0
EPI_LAG = 1
PBUFS = 18
BF = mybir.dt.bfloat16
F32 = mybir.dt.float32
I16 = mybir.dt.int16
BF_NP = ml_dtypes.bfloat16
RG = [list(range(NCORES))]
A = mybir.AluOpType
AF = mybir.ActivationFunctionType

# Schraudolph exp constants (bf16 = bitcast(int16(SCH_A * v + SCH_B)))
SCH_A = 184.6650292  # 2^7 / ln 2
SCH_B = 16256.0 - 7.32  # 127 * 2^7, centered (hw rounds to nearest)

# Query units (hs, he, nch): processed in order u1, u2, u0 per pair so the
# last stats AllReduce covers the 2-chunk unit.
UNITS = [(0, 256, 2), (256, 1152, 7), (1152, 2048, 7)]
PORDER = [1, 2, 0]
NU = len(UNITS)


def _units_for(S_):
    if S_ == 2048:
        return UNITS, PORDER
    nch = S_ // 128
    assert nch <= 7
    return [(0, S_, nch)], [0]

# greedy ACT/DVE balance cost model (ns): per-instr, per-col
ACT_COST = (190.0, 0.833)
DVE_COST = (127.0, 1.042)
# recurring per-pair engine loads not part of the flexible item list
DVE_PAIR_FIXED = float(os.environ.get("K_DVE_PAIR", "3700"))
ACT_PAIR_FIXED = float(os.environ.get("K_ACT_PAIR", "1250"))

LAST_RESULTS = None  # BassKernelResults of the last run (for test harness)


def _exp_schedule(B_=B, S_=S):
    """Greedy ACT/DVE balance over exp tiles and psum->sbuf copies in
    emission order. Returns {key: 'act'|'dve'}."""
    units, porder = _units_for(S_)
    NT_ = S_ // 128
    load = {"act": 0.0, "dve": 0.0}
    sched = {}

    def assign(key, n):
        cost_a = ACT_COST[0] + ACT_COST[1] * n
        cost_d = DVE_COST[0] + DVE_COST[1] * n
        if load["act"] + cost_a <= load["dve"] + cost_d:
            sched[key] = "act"
            load["act"] += cost_a
        else:
            sched[key] = "dve"
            load["dve"] += cost_d

    for pair in range(B_ * HPC):
        b, hh = divmod(pair, HPC)
        load["dve"] += DVE_PAIR_FIXED
        load["act"] += ACT_PAIR_FIXED
        if hh == 0:
            for c in range(S_ // 512):
                assign(("z", b, c), 512)
        for g in range((NT_ + 7) // 8):
            assign(("v", pair, g), 64 * min(8, NT_ - 8 * g))
        for u in porder:
            hs, he, nch = units[u]
            w = he - hs
            for j in range(he // 128):
                lo = max(0, 128 * j - hs)
                assign((pair, u, j), w - lo)
            if hh == 1:
                for i in range(nch):
                    # sum-of-squares: ACT Square+accum vs DVE stt
                    cost_a = 190.0 + 187.0 + 128 * ACT_COST[1]
                    cost_d = DVE_COST[0] + 128 * DVE_COST[1]
                    if load["act"] + cost_a <= load["dve"] + cost_d:
                        sched[("sq", b, u, i)] = "act"
                        load["act"] += cost_a
                    else:
                        sched[("sq", b, u, i)] = "dve"
                        load["dve"] += cost_d
    return sched


def _build_graph(apply_affine: bool, B_: int = B, S_: int = S, rg=None, fake_ar: bool = False) -> bass.Bass:
    nc = bacc.Bacc()
    NT_ = S_ // 128
    if rg is None:
        rg = RG
    sched = _exp_schedule(B_, S_)
    units_, porder_ = _units_for(S_)
    NU_ = len(units_)

    xx = nc.declare_dram_parameter("xx", [B_, DC, S_], BF, isOutput=False)
    xs = nc.declare_dram_parameter("xs", [B_, S_, DC], F32, isOutput=False)
    wpack = nc.declare_dram_parameter("wpack", [DC, DC + DK], BF, isOutput=False)
    # bbp[b, 0] = plain per-key bias (ACT exp bias); bbp[b, 1] = Schraudolph
    bbp = nc.declare_dram_parameter("bbp", [B_, 2, HPC, 128, NT_], F32, isOutput=False)
    if apply_affine:
        gam = nc.declare_dram_parameter("gam", [128, DC], F32, isOutput=False)
        bet = nc.declare_dram_parameter("bet", [128, DC], F32, isOutput=False)
    out = nc.declare_dram_parameter("out", [B_, S_, DC], F32, isOutput=True)
    dbg_y = None
    if os.environ.get("K_DBG_Y"):
        dbg_y = nc.declare_dram_parameter("dbg_y", [B_, 128, S_], F32, isOutput=True)
    dbg_p = None
    if os.environ.get("K_DBG_P"):
        dbg_p = nc.declare_dram_parameter("dbg_p", [NT_, 128, 896], F32, isOutput=True)
        dbg_ch = nc.declare_dram_parameter("dbg_ch", [128, 455], F32, isOutput=True)

    # constants baked into the NEFF: [idn128 | upper-triangle MASKNEG] bf16
    trineg_np = np.where(
        np.arange(128)[:, None] > np.arange(128)[None, :], MASKNEG, 0.0
    ).astype(np.float32)
    imask_h = nc.inline_tensor(
        np.concatenate([np.eye(128, dtype=np.float32), trineg_np], axis=1).astype(BF_NP),
        name="imask",
    )

    # collective bounce buffers: LayerNorm stats per (batch, unit):
    # [2(sum,sumsq), 128 rows, chunks]
    stats_in = nc.dram_tensor("stats_in", [B_, NU_, 2, 128, 7], F32)
    stats_out = nc.dram_tensor("stats_out", [B_, NU_, 2, 128, 7], F32, addr_space="Shared")

    NP = B_ * HPC

    def copy_ps(key, dst, src):
        if sched[key] == "act":
            nc.scalar.activation(dst, src, AF.Copy)
        else:
            nc.vector.tensor_copy(dst, src)

    with TileContext(nc) as tc:
        with (
            tc.tile_pool(name="consts", bufs=1) as cpool,
            tc.tile_pool(name="sb", bufs=2) as sb,
            tc.tile_pool(name="ps", bufs=1, space="PSUM") as ps,
        ):
            # ---- constants (first-needed first) ----
            wp_t = cpool.tile([DC, DC + DK], BF, tag="wp")
            nc.sync.dma_start(out=wp_t[:], in_=wpack[:, :])
            wz_t = wp_t[:][:, 0:DC]
            wv_t = wp_t[:][:, DC : DC + DK]
            imaskq_t = cpool.tile([128, 256], BF, tag="imask")
            nc.gpsimd.dma_start(out=imaskq_t[:], in_=imask_h[:, :])
            idn128_t = imaskq_t[:][:, 0:128]
            maskt_t = imaskq_t[:][:, 128:256]
            eps_t = cpool.tile([128, 1], F32, tag="eps")
            nc.vector.memset(eps_t[:], EPS)
            if apply_affine:
                gam_t = cpool.tile([128, DC], F32, tag="gam")
                nc.gpsimd.dma_start(out=gam_t[:], in_=gam[:, :])
                bet_t = cpool.tile([128, DC], F32, tag="bet")
                nc.gpsimd.dma_start(out=bet_t[:], in_=bet[:, :])

            y_tiles = {}
            bstate = {}
            pstate = {}
            pending_ln = []

            def _emit_stats(b, u, y_b, accs):
                hs, he, nch = units_[u]
                t0 = hs // 128
                pk = sb.tile([128, 14], F32, tag="pk", bufs=3)
                nc.gpsimd.tensor_add(
                    pk[:, 0:nch], accs[0][:, t0 : t0 + nch], accs[1][:, t0 : t0 + nch]
                )
                for i in range(t0, t0 + nch):
                    scr = sb.tile([128, 128], F32, tag="scr")
                    if sched[("sq", b, u, i - t0)] == "act":
                        nc.scalar.activation(
                            scr[:],
                            y_b[:, 128 * i : 128 * i + 128],
                            AF.Square,
                            accum_out=pk[:, nch + i - t0 : nch + 1 + i - t0],
                        )
                    else:
                        nc.vector.scalar_tensor_tensor(
                            scr[:],
                            y_b[:, 128 * i : 128 * i + 128],
                            1.0,
                            y_b[:, 128 * i : 128 * i + 128],
                            A.mult,
                            A.mult,
                            accum_out=pk[:, nch + i - t0 : nch + 1 + i - t0],
                        )
                nc.sync.dma_start(
                    out=stats_in[b, u, :, :, 0:nch].rearrange("c p t -> p c t"),
                    in_=pk[:, 0 : 2 * nch].rearrange("p (c t) -> p c t", t=nch),
                )
                if dbg_y is not None:
                    nc.sync.dma_start(
                        out=dbg_y[b, :, hs:he], in_=y_b[:, hs:he]
                    )
                if fake_ar:
                    nc.sync.dma_start(
                        out=stats_out[b, u, :, :, 0:nch], in_=stats_in[b, u, :, :, 0:nch]
                    )
                else:
                    nc.gpsimd.collective_compute(
                        "AllReduce",
                        A.add,
                        replica_groups=rg,
                        ins=[stats_in[b, u].opt()],
                        outs=[stats_out[b, u].opt()],
                    )
                pending_ln.append(lambda b=b, u=u: emit_ln(b, u, y_b))

            def emit_ln(b, u, y_b):
                hs, he, nch = units_[u]
                t0 = hs // 128
                red = sb.tile([128, 14], F32, tag="red", bufs=3)
                nc.sync.dma_start(
                    out=red[:, 0 : 2 * nch].rearrange("p (c t) -> p c t", t=nch),
                    in_=stats_out[b, u, :, :, 0:nch].rearrange("c p t -> p c t"),
                )
                mean = sb.tile([128, 7], F32, tag="mean", bufs=3)
                nc.gpsimd.tensor_scalar(mean[:, 0:nch], red[:, 0:nch], 1.0 / D, None, A.mult)
                msq = sb.tile([128, 7], F32, tag="msq", bufs=3)
                nc.gpsimd.tensor_mul(msq[:, 0:nch], mean[:, 0:nch], mean[:, 0:nch])
                var = sb.tile([128, 7], F32, tag="var", bufs=3)
                nc.gpsimd.tensor_scalar(
                    var[:, 0:nch], red[:, nch : 2 * nch], 1.0 / D, None, A.mult
                )
                nc.gpsimd.tensor_tensor(
                    var[:, 0:nch], var[:, 0:nch], msq[:, 0:nch], A.subtract
                )
                lnv = sb.tile([128, 7], F32, tag="lnv", bufs=3)
                nc.scalar.activation(lnv[:, 0:nch], var[:, 0:nch], AF.Ln, bias=eps_t[:])
                rstd = sb.tile([128, 7], F32, tag="rstd", bufs=3)
                nc.scalar.activation(rstd[:, 0:nch], lnv[:, 0:nch], AF.Exp, scale=-0.5)
                ostb = sb.tile([128, 128 * 7], F32, tag="ost", bufs=2)
                for k in range(nch):
                    i = t0 + k
                    nc.gpsimd.tensor_scalar(
                        ostb[:, 128 * k : 128 * k + 128],
                        y_b[:, 128 * i : 128 * i + 128],
                        mean[:, k : k + 1],
                        rstd[:, k : k + 1],
                        A.subtract,
                        A.mult,
                    )
                    if apply_affine:
                        nc.gpsimd.tensor_mul(
                            ostb[:, 128 * k : 128 * k + 128],
                            ostb[:, 128 * k : 128 * k + 128],
                            gam_t[:],
                        )
                        nc.gpsimd.tensor_add(
                            ostb[:, 128 * k : 128 * k + 128],
                            ostb[:, 128 * k : 128 * k + 128],
                            bet_t[:],
                        )
                nc.sync.dma_start(
                    out=out[b, 128 * t0 : 128 * (t0 + nch), :].rearrange(
                        "(i p) d -> p i d", p=128
                    ),
                    in_=ostb[:, 0 : 128 * nch].rearrange("p (i d) -> p i d", d=128),
                )

            def emit_proj(pair):
                b, hh = divmod(pair, HPC)
                if hh == 0:
                    xx_b = sb.tile([128, S_], BF, tag="xx", name=f"xx{b}", bufs=2)
                    nc.sync.dma_start(out=xx_b[:, 0 : S_ // 2], in_=xx[b, :, 0 : S_ // 2])
                    nc.sync.dma_start(out=xx_b[:, S_ // 2 : S_], in_=xx[b, :, S_ // 2 : S_])
                    bb_t = sb.tile([128, 2 * HPC * NT_], F32, tag="bb", name=f"bb{b}", bufs=2)
                    nc.sync.dma_start(
                        out=bb_t[:].rearrange("p (v h j) -> p v h j", v=2, h=HPC),
                        in_=bbp[b].rearrange("v h p j -> p v h j"),
                    )
                    # zz projection: both heads at once via block-diag wz
                    zz_b = sb.tile([128, S_], BF, tag="zz", name=f"zz{b}", bufs=2)
                    for c in range(S_ // 512):
                        zp = ps.tile([128, 512], F32, tag="op", bufs=2, name=f"zp{b}_{c}")
                        nc.tensor.matmul(
                            zp[:],
                            lhsT=wz_t,
                            rhs=xx_b[:, 512 * c : 512 * c + 512],
                            start=True,
                            stop=True,
                        )
                        copy_ps(("z", b, c), zz_b[:, 512 * c : 512 * c + 512], zp[:])
                    xs_b = sb.tile([128, S_], F32, tag="xs", name=f"xs{b}")
                    nc.sync.dma_start(
                        out=xs_b[:].rearrange("p (i d) -> p i d", d=128),
                        in_=xs[b].rearrange("(i p) d -> p i d", p=128),
                    )
                    y_b = sb.tile([128, S_], F32, tag="y", name=f"y{b}", bufs=3)
                    y_tiles[b] = y_b
                    bstate[b] = (xx_b, zz_b, xs_b, y_b, bb_t, {})
                xx_b, zz_b, xs_b, y_b, bb_t, accs = bstate[b]
                # V projection for this head: v[t, 65j:65j+64], ones at col 64
                v_t = sb.tile([128, NT_ * 65], BF, tag="v", name=f"v{pair}", bufs=3)
                v3 = v_t[:].rearrange("p (t w) -> p t w", w=65)
                nc.gpsimd.memset(v3[:, :, 64:65], 1.0)
                for g in range((NT_ + 7) // 8):
                    gn = min(8, NT_ - 8 * g)
                    vp = ps.tile([128, 512], F32, tag="op", bufs=2, name=f"vp{pair}_{g}")
                    for uu in range(gn):
                        j = 8 * g + uu
                        nc.tensor.matmul(
                            vp[:, DK * uu : DK * uu + DK],
                            lhsT=xx_b[:][DK * hh : DK * hh + DK, 128 * j : 128 * j + 128],
                            rhs=wv_t[DK * hh : DK * hh + DK, :],
                            start=True,
                            stop=True,
                        )
                    copy_ps(
                        ("v", pair, g),
                        v3[:, 8 * g : 8 * g + gn, 0:64],
                        vp[:, 0 : DK * gn].rearrange("q (t w) -> q t w", w=DK),
                    )
                acc_h = sb.tile([128, NT_], F32, tag=f"acc{hh}", name=f"acc{pair}", bufs=2)
                accs[hh] = acc_h
                pstate[pair] = (v3, acc_h)

            def emit_junit(pair, u, hooks=None):
                """Score/exp/PV loop for one query unit; returns the deferred
                normalize epilogue closure."""
                b, hh = divmod(pair, HPC)
                hs, he, nch = units_[u]
                w = he - hs
                v3, acc_h = pstate[pair]
                xx_b, zz_b, xs_b, y_b, bb_t, accs = bstate[b]
                hooks = dict(hooks or {})
                xh = xx_b[:][DK * hh : DK * hh + DK, :]
                zh = zz_b[:][DK * hh : DK * hh + DK, :]
                ch_t = ps.tile([128, 455], F32, tag="ch", bufs=2)
                pviews = []
                prev_pv = None
                for j in range(he // 128):
                    s0 = 128 * j
                    rel = s0 - hs
                    lo = max(0, rel)
                    sp = ps.tile([128, 896], F32, tag="sp", bufs=2)
                    # score matmuls, split at the col-512 psum bank boundary
                    def score_span(cs, ce):
                        while cs < ce:
                            sl = min(512 * (cs // 512) + 512, ce) - cs
                            nc.tensor.matmul(
                                sp[:, cs : cs + sl],
                                lhsT=xh[:, s0 : s0 + 128],
                                rhs=zh[:, hs + cs : hs + cs + sl],
                                start=True,
                                stop=True,
                                skip_group_check=True,
                            )
                            cs += sl
                    if rel < 0:
                        score_span(0, w)
                    else:
                        nc.tensor.matmul(
                            sp[:, rel : rel + 128],
                            lhsT=idn128_t,
                            rhs=maskt_t,
                            start=True,
                            stop=False,
                            skip_group_check=True,
                        )
                        nc.tensor.matmul(
                            sp[:, rel : rel + 128],
                            lhsT=xh[:, s0 : s0 + 128],
                            rhs=zh[:, s0 : s0 + 128],
                            start=False,
                            stop=True,
                            skip_group_check=True,
                        )
                        score_span(rel + 128, w)
                    bcol = NT_ * hh + j
                    if sched[(pair, u, j)] == "act":
                        p = sb.tile([128, 896], BF, tag="pa", bufs=PBUFS)
                        nc.scalar.activation(
                            p[:, lo:w], sp[:, lo:w], AF.Exp,
                            bias=bb_t[:, bcol : bcol + 1],
                        )
                        pview = p[:]
                    else:
                        p = sb.tile([128, 896], I16, tag="pi", bufs=PBUFS)
                        nc.vector.tensor_scalar(
                            p[:, lo:w], sp[:, lo:w],
                            SCH_A,
                            bb_t[:, 2 * NT_ + bcol : 2 * NT_ + bcol + 1],
                            A.mult, A.add,
                        )
                        pview = p[:].bitcast(BF)

                    if dbg_p is not None and pair == 0:
                        pf = sb.tile([128, 896], F32, tag="pf", bufs=2)
                        nc.vector.tensor_copy(pf[:, lo:w], pview[:, lo:w])
                        nc.sync.dma_start(out=dbg_p[j, :, lo:w], in_=pf[:, lo:w])
                    pviews.append(pview)

                    # PSUM allows one open accumulation group per bank, so a
                    # chunk's PV contributions are emitted as one contiguous
                    # open->close burst once its diagonal P tile exists;
                    # deferred by one j so PE isn't gated on exp latency.
                    def _burst(c=j - hs // 128):
                        for jj in range(hs // 128 + c + 1):
                            nc.tensor.matmul(
                                ch_t[:, 65 * c : 65 * c + 65],
                                lhsT=pviews[jj][:, 128 * c : 128 * c + 128],
                                rhs=v3[:, jj, :],
                                start=(jj == 0),
                                stop=(jj == hs // 128 + c),
                                skip_group_check=True,
                            )

                    if prev_pv is not None:
                        prev_pv()
                    prev_pv = _burst if rel >= 0 else None
                    if j in hooks:
                        hooks.pop(j)()
                if prev_pv is not None:
                    prev_pv()
                for hk in hooks.values():  # unit shorter than hook points
                    hk()

                def _epilogue():
                    t0 = hs // 128
                    if dbg_p is not None and pair == 0:
                        chf = sb.tile([128, 455], F32, tag="chf", bufs=2)
                        nc.vector.tensor_copy(chf[:, 0 : 65 * nch], ch_t[:, 0 : 65 * nch])
                        nc.sync.dma_start(out=dbg_ch[:, 0 : 65 * nch], in_=chf[:, 0 : 65 * nch])
                    r7 = sb.tile([128, 7], F32, tag="r7", bufs=3)
                    nc.vector.reciprocal(
                        r7[:, 0:nch],
                        ch_t[:].rearrange("p (c w) -> p c w", w=65)[:, 0:nch, 64:65],
                    )
                    for c in range(nch):
                        i = t0 + c
                        nc.vector.scalar_tensor_tensor(
                            y_b[:, 128 * i + DK * hh : 128 * i + DK * hh + DK],
                            ch_t[:, 65 * c : 65 * c + 64],
                            r7[:, c : c + 1],
                            xs_b[:, 128 * i + DK * hh : 128 * i + DK * hh + DK],
                            A.mult,
                            A.add,
                            accum_out=acc_h[:, i : i + 1],
                        )
                    if hh == HPC - 1:
                        _emit_stats(b, u, y_b, accs)

                return _epilogue

            emit_proj(0)
            pending = []
            for pair in range(NP):
                for k, u in enumerate(porder_):
                    hooks = {}
                    if k == 0:
                        if pair + 1 < NP:
                            hooks[3] = (lambda pr=pair + 1: emit_proj(pr))
                        if pending_ln and pair >= 3:
                            hooks[6] = pending_ln.pop(0)
                    elif pending_ln and pair >= 3:
                        hooks[2] = pending_ln.pop(0)
                    epi = emit_junit(pair, u, hooks=hooks)
                    pending.append(epi)
                    if len(pending) > EPI_LAG:
                        pending.pop(0)()
            for e in pending:
                e()
            while pending_ln:
                pending_ln.pop(0)()

    # Restrict Exp/Ln/Copy to the shared natural_log_exp_and_others table set
    # so the whole kernel uses one ACT table load.
    import concourse.bacc as _bacc_mod

    _orig_tables = _bacc_mod.get_activation_tables

    def _filtered_tables(arch):
        outm = {}
        for name, fns in _orig_tables(arch).items():
            if name != "natural_log_exp_and_others":
                fns = set(fns) - {AF.Exp, AF.Ln, AF.Copy, AF.Square}
            outm[name] = fns
        return outm

    _bacc_mod.get_activation_tables = _filtered_tables
    try:
        nc.compile()
    finally:
        _bacc_mod.get_activation_tables = _orig_tables
    return nc


_GRAPH_CACHE = {}


def _get_graph(apply_affine: bool) -> bass.Bass:
    if apply_affine not in _GRAPH_CACHE:
        _GRAPH_CACHE[apply_affine] = _build_graph(apply_affine)
    return _GRAPH_CACHE[apply_affine]


def _prep_in_maps(x, Wq, bq, Wk, bk, Wv, bv, gamma, beta, apply_affine):
    scale = 1.0 / np.sqrt(np.float64(DK))
    in_maps = []
    for i in range(NCORES):
        dsl = slice(DC * i, DC * (i + 1))
        hsl = slice(HPC * i, HPC * (i + 1))
        x_sl = x[:, :, dsl]                       # [B, S, 128]
        xx_np = x_sl.transpose(0, 2, 1)           # [B, 128, S]
        Wq_h = Wq[hsl].astype(np.float64)
        bq_h = bq[hsl].astype(np.float64)
        Wk_h = Wk[hsl].astype(np.float64)
        # A_h = Wk Wq^T * scale ; z = A x_s ; score += (Wk bq * scale) . x_t
        A_h = np.einsum("hde,hfe->hdf", Wk_h, Wq_h) * scale   # [h, dK, dQ]
        u_h = np.einsum("hde,he->hd", Wk_h, bq_h) * scale     # [h, dK]
        wz = np.zeros((DC, DC), np.float64)
        for hh in range(HPC):
            blk = slice(DK * hh, DK * hh + DK)
            wz[blk, blk] = A_h[hh].T
        wv = np.zeros((DC, DK), np.float64)
        for hh in range(HPC):
            wv[DK * hh : DK * hh + DK, :] = Wv[hsl][hh]
        # per-key bias bb[b, hh, t] = u_h . x_h[:, t]
        bb = np.einsum("hd,bthd->bht", u_h,
                       x_sl.reshape(x.shape[0], x.shape[1], HPC, DK).astype(np.float64))
        bbq_np = bb.reshape(x.shape[0], HPC, S // 128, 128).transpose(0, 1, 3, 2)
        bbs_np = bbq_np * SCH_A + SCH_B
        bbp_np = np.stack([bbq_np, bbs_np], axis=1)  # [B, 2, HPC, 128, NT]
        xs_np = x_sl + bv[hsl].reshape(1, 1, DC)
        m = {
            "xx": np.ascontiguousarray(xx_np).astype(BF_NP),
            "xs": np.ascontiguousarray(xs_np).astype(np.float32),
            "wpack": np.ascontiguousarray(
                np.concatenate([wz, wv], axis=1)
            ).astype(BF_NP),
            "bbp": np.ascontiguousarray(bbp_np).astype(np.float32),
        }
        if apply_affine:
            m["gam"] = np.ascontiguousarray(
                np.tile(gamma[dsl][None, :], (128, 1))
            ).astype(np.float32)
            m["bet"] = np.ascontiguousarray(
                np.tile(beta[dsl][None, :], (128, 1))
            ).astype(np.float32)
        in_maps.append(m)
    return in_maps


def kernel(x, Wq, bq, Wk, bk, Wv, bv, gamma, beta):
    global LAST_RESULTS
    x = np.asarray(x, np.float32)
    Wq = np.asarray(Wq, np.float32)
    bq = np.asarray(bq, np.float32)
    Wk = np.asarray(Wk, np.float32)
    bk = np.asarray(bk, np.float32)
    Wv = np.asarray(Wv, np.float32)
    bv = np.asarray(bv, np.float32)
    gamma = np.asarray(gamma, np.float32)
    beta = np.asarray(beta, np.float32)

    apply_affine = not (
        np.allclose(gamma, 1.0, atol=0.0, rtol=0.0)
        and np.allclose(beta, 0.0, atol=0.0, rtol=0.0)
    )
    fake_ar = bool(int(os.environ.get("KERNEL_FAKE_AR", "0")))
    nc = _get_graph(apply_affine) if not fake_ar else _build_graph(apply_affine, fake_ar=True)

    in_maps = _prep_in_maps(x, Wq, bq, Wk, bk, Wv, bv, gamma, beta, apply_affine)

    res = run_bass_kernel_spmd(
        nc,
        in_maps,
        core_ids=list(range(NCORES)),
        trace=bool(int(os.environ.get("KERNEL_TRACE", "0"))),
    )
    LAST_RESULTS = res
    outs = [np.asarray(r["out"], np.float32) for r in res.results]
    return np.concatenate(outs, axis=2)


if __name__ == "__main__":
    nc = _build_graph(False)
    print("graph built ok:", len(nc.inst_map), "instructions")


# revision 20
# speedup vs baseline: 1.0521x; 1.0147x over previous
"""Trainium2 Bass kernel: per-head attention + residual + LayerNorm.

Problem shape: x [4, 2048, 1024], 16 heads of dk=64, causal softmax attention
with per-head Q/K/V linear projections, residual add, LayerNorm(D).

Sharding (8 cores): head-parallel. Core i owns heads (2i, 2i+1) = feature
columns 128*i : 128*(i+1). Only cross-core traffic: per-(batch,unit) AllReduce
of LayerNorm partial sums.

v2 design (vs the 202us baseline, whose bottleneck was the ACT engine at 77%
busy running exp over the causal score area):
- Softmax-invariance scores: sp[t,s] = x_t^T (Wk Wq^T/sqrt(dk)) x_s + u.x_t
  (query-only bias terms cancel in softmax). The per-key bias u.x_t is
  computed on the HOST and folded into the exp instruction's per-partition
  bias operand - no bias-row augmentation, no on-device bias adds.
- zz = block-diag(A_h0^T, A_h1^T) @ xx projects BOTH heads in one matmul
  stream ([128,S]); per-head scores contract over a 64-partition slice.
- exp is split across the ACT engine (true Exp) and DVE (Schraudolph:
  bf16 = bitcast(int16(184.665*s + bias)), a single tensor_scalar with
  ~1.8% sigma error that mostly cancels in softmax). A build-time greedy
  balancer also assigns the zz/v PSUM->SBUF copies to ACT (AF.Copy, same
  act table) or DVE. GPSIMD cannot touch PSUM, so Pool instead absorbs the
  SBUF-only LayerNorm/stats elementwise work.
- PV reoriented to P^T V: out[s-chunk 128, 65] with lhsT = P chunk; free
  size 65 per matmul (vs 512) halves PE PV time, writes y directly (no PE
  transposes, no O^T drain copies), and the rhs ones-column lands the
  softmax denominator in chunk col 64. V bias is pre-added into xs on host.
- Query-units of (896, 896, 256) cols keep the PV chunk tile (nch x 65 fp32
  <= 455) inside one PSUM bank; processed [u1, u2, u0] so the last
  (batch,unit) stats AllReduce covers the smallest unit (short tail).
- LayerNorm output (emit_ln) is emitted ~one pair after its AllReduce
  fires, spreading Pool/out-DMA work over compute instead of a serial tail.

Self-contained: hardcodes all shapes; no sibling imports.
"""

import os
import numpy as np
import ml_dtypes

import concourse.bass as bass
import concourse.bacc as bacc
import concourse.mybir as mybir
from concourse.tile import TileContext
from concourse.bass_utils import run_bass_kernel_spmd

B, S, D, H = 4, 2048, 1024, 16
NCORES = 8
HPC = H // NCORES          # heads per core = 2
DK = D // H                # 64
DC = HPC * DK              # 128 feature cols per core
NT = S // 128              # 16 row tiles of 128
EPS = 1e-5
MASKNEG = -40.0
EPI_LAG = 1
PV_LAG = int(os.environ.get("K_PV_LAG", "3"))
PBUFS = 18
BF = mybir.dt.bfloat16
F32 = mybir.dt.float32
I16 = mybir.dt.int16
BF_NP = ml_dtypes.bfloat16
RG = [list(range(NCORES))]
A = mybir.AluOpType
AF = mybir.ActivationFunctionType

# Schraudolph exp constants (bf16 = bitcast(int16(SCH_A * v + SCH_B)))
SCH_A = 184.6650292  # 2^7 / ln 2
SCH_B = 16256.0 - 7.32  # 127 * 2^7, centered (hw rounds to nearest)

# Query units (hs, he, nch): processed in order u1, u2, u0 per pair so the
# last stats AllReduce covers the 2-chunk unit.
UNITS = [(0, 256, 2), (256, 1152, 7), (1152, 2048, 7)]
PORDER = [1, 2, 0]
NU = len(UNITS)


def _units_for(S_):
    if S_ == 2048:
        return UNITS, PORDER
    nch = S_ // 128
    assert nch <= 7
    return [(0, S_, nch)], [0]

# greedy ACT/DVE balance cost model (ns): per-instr, per-col
ACT_COST = (190.0, 0.833)
DVE_COST = (127.0, 1.042)
# recurring per-pair engine loads not part of the flexible item list
DVE_PAIR_FIXED = float(os.environ.get("K_DVE_PAIR", "3700"))
ACT_PAIR_FIXED = float(os.environ.get("K_ACT_PAIR", "1250"))

LAST_RESULTS = None  # BassKernelResults of the last run (for test harness)


def _exp_schedule(B_=B, S_=S):
    """Greedy ACT/DVE balance over exp tiles and psum->sbuf copies in
    emission order. Returns {key: 'act'|'dve'}."""
    units, porder = _units_for(S_)
    NT_ = S_ // 128
    load = {"act": 0.0, "dve": 0.0}
    sched = {}

    def assign(key, n):
        cost_a = ACT_COST[0] + ACT_COST[1] * n
        cost_d = DVE_COST[0] + DVE_COST[1] * n
        if load["act"] + cost_a <= load["dve"] + cost_d:
            sched[key] = "act"
            load["act"] += cost_a
        else:
            sched[key] = "dve"
            load["dve"] += cost_d

    for pair in range(B_ * HPC):
        b, hh = divmod(pair, HPC)
        load["dve"] += DVE_PAIR_FIXED
        load["act"] += ACT_PAIR_FIXED
        if hh == 0:
            for c in range(S_ // 512):
                assign(("z", b, c), 512)
        for g in range((NT_ + 7) // 8):
            assign(("v", pair, g), 64 * min(8, NT_ - 8 * g))
        for u in porder:
            hs, he, nch = units[u]
            w = he - hs
            for j in range(he // 128):
                lo = max(0, 128 * j - hs)
                assign((pair, u, j), w - lo)
            if hh == 1:
                for i in range(nch):
                    # sum-of-squares: ACT Square+accum vs DVE stt
                    cost_a = 190.0 + 187.0 + 128 * ACT_COST[1]
                    cost_d = DVE_COST[0] + 128 * DVE_COST[1]
                    if load["act"] + cost_a <= load["dve"] + cost_d:
                        sched[("sq", b, u, i)] = "act"
                        load["act"] += cost_a
                    else:
                        sched[("sq", b, u, i)] = "dve"
                        load["dve"] += cost_d
    return sched


def _build_graph(apply_affine: bool, B_: int = B, S_: int = S, rg=None, fake_ar: bool = False) -> bass.Bass:
    nc = bacc.Bacc()
    NT_ = S_ // 128
    if rg is None:
        rg = RG
    sched = _exp_schedule(B_, S_)
    units_, porder_ = _units_for(S_)
    NU_ = len(units_)

    xx = nc.declare_dram_parameter("xx", [B_, DC, S_], BF, isOutput=False)
    xs = nc.declare_dram_parameter("xs", [B_, S_, DC], F32, isOutput=False)
    wpack = nc.declare_dram_parameter("wpack", [DC, DC + DK], BF, isOutput=False)
    # bbp[b, 0] = plain per-key bias (ACT exp bias); bbp[b, 1] = Schraudolph
    bbp = nc.declare_dram_parameter("bbp", [B_, 2, HPC, 128, NT_], F32, isOutput=False)
    if apply_affine:
        gam = nc.declare_dram_parameter("gam", [128, DC], F32, isOutput=False)
        bet = nc.declare_dram_parameter("bet", [128, DC], F32, isOutput=False)
    out = nc.declare_dram_parameter("out", [B_, S_, DC], F32, isOutput=True)
    dbg_y = None
    if os.environ.get("K_DBG_Y"):
        dbg_y = nc.declare_dram_parameter("dbg_y", [B_, 128, S_], F32, isOutput=True)
    dbg_p = None
    if os.environ.get("K_DBG_P"):
        dbg_p = nc.declare_dram_parameter("dbg_p", [NT_, 128, 896], F32, isOutput=True)
        dbg_ch = nc.declare_dram_parameter("dbg_ch", [128, 455], F32, isOutput=True)

    # constants baked into the NEFF: [idn128 | upper-triangle MASKNEG] bf16
    trineg_np = np.where(
        np.arange(128)[:, None] > np.arange(128)[None, :], MASKNEG, 0.0
    ).astype(np.float32)
    imask_h = nc.inline_tensor(
        np.concatenate([np.eye(128, dtype=np.float32), trineg_np], axis=1).astype(BF_NP),
        name="imask",
    )

    # collective bounce buffers: LayerNorm stats per (batch, unit):
    # [2(sum,sumsq), 128 rows, chunks]
    stats_in = nc.dram_tensor("stats_in", [B_, NU_, 2, 128, 7], F32)
    stats_out = nc.dram_tensor("stats_out", [B_, NU_, 2, 128, 7], F32, addr_space="Shared")

    NP = B_ * HPC

    def copy_ps(key, dst, src):
        if sched[key] == "act":
            nc.scalar.activation(dst, src, AF.Copy)
        else:
            nc.vector.tensor_copy(dst, src)

    with TileContext(nc) as tc:
        with (
            tc.tile_pool(name="consts", bufs=1) as cpool,
            tc.tile_pool(name="sb", bufs=2) as sb,
            tc.tile_pool(name="ps", bufs=1, space="PSUM") as ps,
        ):
            # ---- constants (first-needed first) ----
            wp_t = cpool.tile([DC, DC + DK], BF, tag="wp")
            nc.sync.dma_start(out=wp_t[:], in_=wpack[:, :])
            wz_t = wp_t[:][:, 0:DC]
            wv_t = wp_t[:][:, DC : DC + DK]
            imaskq_t = cpool.tile([128, 256], BF, tag="imask")
            nc.gpsimd.dma_start(out=imaskq_t[:], in_=imask_h[:, :])
            idn128_t = imaskq_t[:][:, 0:128]
            maskt_t = imaskq_t[:][:, 128:256]
            eps_t = cpool.tile([128, 1], F32, tag="eps")
            nc.vector.memset(eps_t[:], EPS)
            if apply_affine:
                gam_t = cpool.tile([128, DC], F32, tag="gam")
                nc.gpsimd.dma_start(out=gam_t[:], in_=gam[:, :])
                bet_t = cpool.tile([128, DC], F32, tag="bet")
                nc.gpsimd.dma_start(out=bet_t[:], in_=bet[:, :])

            y_tiles = {}
            bstate = {}
            pstate = {}
            pending_ln = []

            def _emit_stats(b, u, y_b, accs):
                hs, he, nch = units_[u]
                t0 = hs // 128
                pk = sb.tile([128, 14], F32, tag="pk", bufs=3)
                nc.gpsimd.tensor_add(
                    pk[:, 0:nch], accs[0][:, t0 : t0 + nch], accs[1][:, t0 : t0 + nch]
                )
                for i in range(t0, t0 + nch):
                    scr = sb.tile([128, 128], F32, tag="scr")
                    if sched[("sq", b, u, i - t0)] == "act":
                        nc.scalar.activation(
                            scr[:],
                            y_b[:, 128 * i : 128 * i + 128],
                            AF.Square,
                            accum_out=pk[:, nch + i - t0 : nch + 1 + i - t0],
                        )
                    else:
                        nc.vector.scalar_tensor_tensor(
                            scr[:],
                            y_b[:, 128 * i : 128 * i + 128],
                            1.0,
                            y_b[:, 128 * i : 128 * i + 128],
                            A.mult,
                            A.mult,
                            accum_out=pk[:, nch + i - t0 : nch + 1 + i - t0],
                        )
                nc.sync.dma_start(
                    out=stats_in[b, u, :, :, 0:nch].rearrange("c p t -> p c t"),
                    in_=pk[:, 0 : 2 * nch].rearrange("p (c t) -> p c t", t=nch),
                )
                if dbg_y is not None:
                    nc.sync.dma_start(
                        out=dbg_y[b, :, hs:he], in_=y_b[:, hs:he]
                    )
                if fake_ar:
                    nc.sync.dma_start(
                        out=stats_out[b, u, :, :, 0:nch], in_=stats_in[b, u, :, :, 0:nch]
                    )
                else:
                    nc.gpsimd.collective_compute(
                        "AllReduce",
                        A.add,
                        replica_groups=rg,
                        ins=[stats_in[b, u].opt()],
                        outs=[stats_out[b, u].opt()],
                    )
                pending_ln.append(lambda b=b, u=u: emit_ln(b, u, y_b))

            def emit_ln(b, u, y_b):
                hs, he, nch = units_[u]
                t0 = hs // 128
                red = sb.tile([128, 14], F32, tag="red", bufs=3)
                nc.sync.dma_start(
                    out=red[:, 0 : 2 * nch].rearrange("p (c t) -> p c t", t=nch),
                    in_=stats_out[b, u, :, :, 0:nch].rearrange("c p t -> p c t"),
                )
                mean = sb.tile([128, 7], F32, tag="mean", bufs=3)
                nc.gpsimd.tensor_scalar(mean[:, 0:nch], red[:, 0:nch], 1.0 / D, None, A.mult)
                msq = sb.tile([128, 7], F32, tag="msq", bufs=3)
                nc.gpsimd.tensor_mul(msq[:, 0:nch], mean[:, 0:nch], mean[:, 0:nch])
                var = sb.tile([128, 7], F32, tag="var", bufs=3)
                nc.gpsimd.tensor_scalar(
                    var[:, 0:nch], red[:, nch : 2 * nch], 1.0 / D, None, A.mult
                )
                nc.gpsimd.tensor_tensor(
                    var[:, 0:nch], var[:, 0:nch], msq[:, 0:nch], A.subtract
                )
                lnv = sb.tile([128, 7], F32, tag="lnv", bufs=3)
                nc.scalar.activation(lnv[:, 0:nch], var[:, 0:nch], AF.Ln, bias=eps_t[:])
                rstd = sb.tile([128, 7], F32, tag="rstd", bufs=3)
                nc.scalar.activation(rstd[:, 0:nch], lnv[:, 0:nch], AF.Exp, scale=-0.5)
                ostb = sb.tile([128, 128 * 7], F32, tag="ost", bufs=2)
                for k in range(nch):
                    i = t0 + k
                    nc.gpsimd.tensor_scalar(
                        ostb[:, 128 * k : 128 * k + 128],
                        y_b[:, 128 * i : 128 * i + 128],
                        mean[:, k : k + 1],
                        rstd[:, k : k + 1],
                        A.subtract,
                        A.mult,
                    )
                    if apply_affine:
                        nc.gpsimd.tensor_mul(
                            ostb[:, 128 * k : 128 * k + 128],
                            ostb[:, 128 * k : 128 * k + 128],
                            gam_t[:],
                        )
                        nc.gpsimd.tensor_add(
                            ostb[:, 128 * k : 128 * k + 128],
                            ostb[:, 128 * k : 128 * k + 128],
                            bet_t[:],
                        )
                nc.sync.dma_start(
                    out=out[b, 128 * t0 : 128 * (t0 + nch), :].rearrange(
                        "(i p) d -> p i d", p=128
                    ),
                    in_=ostb[:, 0 : 128 * nch].rearrange("p (i d) -> p i d", d=128),
                )

            def emit_proj(pair):
                b, hh = divmod(pair, HPC)
                if hh == 0:
                    xx_b = sb.tile([128, S_], BF, tag="xx", name=f"xx{b}", bufs=2)
                    nc.sync.dma_start(out=xx_b[:, 0 : S_ // 2], in_=xx[b, :, 0 : S_ // 2])
                    nc.sync.dma_start(out=xx_b[:, S_ // 2 : S_], in_=xx[b, :, S_ // 2 : S_])
                    bb_t = sb.tile([128, 2 * HPC * NT_], F32, tag="bb", name=f"bb{b}", bufs=2)
                    nc.sync.dma_start(
                        out=bb_t[:].rearrange("p (v h j) -> p v h j", v=2, h=HPC),
                        in_=bbp[b].rearrange("v h p j -> p v h j"),
                    )
                    # zz projection: both heads at once via block-diag wz
                    zz_b = sb.tile([128, S_], BF, tag="zz", name=f"zz{b}", bufs=2)
                    for c in range(S_ // 512):
                        zp = ps.tile([128, 512], F32, tag="op", bufs=2, name=f"zp{b}_{c}")
                        nc.tensor.matmul(
                            zp[:],
                            lhsT=wz_t,
                            rhs=xx_b[:, 512 * c : 512 * c + 512],
                            start=True,
                            stop=True,
                        )
                        copy_ps(("z", b, c), zz_b[:, 512 * c : 512 * c + 512], zp[:])
                    xs_b = sb.tile([128, S_], F32, tag="xs", name=f"xs{b}")
                    nc.sync.dma_start(
                        out=xs_b[:].rearrange("p (i d) -> p i d", d=128),
                        in_=xs[b].rearrange("(i p) d -> p i d", p=128),
                    )
                    y_b = sb.tile([128, S_], F32, tag="y", name=f"y{b}", bufs=3)
                    y_tiles[b] = y_b
                    bstate[b] = (xx_b, zz_b, xs_b, y_b, bb_t, {})
                xx_b, zz_b, xs_b, y_b, bb_t, accs = bstate[b]
                acc_h = sb.tile([128, NT_], F32, tag=f"acc{hh}", name=f"acc{pair}", bufs=2)
                accs[hh] = acc_h
                pstate[pair] = [None, acc_h]

            def emit_proj_v(pair):
                b, hh = divmod(pair, HPC)
                xx_b, zz_b, xs_b, y_b, bb_t, accs = bstate[b]
                # V projection for this head: v[t, 65j:65j+64], ones at col 64
                v_t = sb.tile([128, NT_ * 65], BF, tag="v", name=f"v{pair}", bufs=3)
                v3 = v_t[:].rearrange("p (t w) -> p t w", w=65)
                nc.gpsimd.memset(v3[:, :, 64:65], 1.0)
                for g in range((NT_ + 7) // 8):
                    gn = min(8, NT_ - 8 * g)
                    vp = ps.tile([128, 512], F32, tag="op", bufs=2, name=f"vp{pair}_{g}")
                    for uu in range(gn):
                        j = 8 * g + uu
                        nc.tensor.matmul(
                            vp[:, DK * uu : DK * uu + DK],
                            lhsT=xx_b[:][DK * hh : DK * hh + DK, 128 * j : 128 * j + 128],
                            rhs=wv_t[DK * hh : DK * hh + DK, :],
                            start=True,
                            stop=True,
                        )
                    copy_ps(
                        ("v", pair, g),
                        v3[:, 8 * g : 8 * g + gn, 0:64],
                        vp[:, 0 : DK * gn].rearrange("q (t w) -> q t w", w=DK),
                    )
                pstate[pair][0] = v3

            def emit_junit(pair, u, hooks=None):
                """Score/exp/PV loop for one query unit; returns the deferred
                normalize epilogue closure."""
                b, hh = divmod(pair, HPC)
                hs, he, nch = units_[u]
                w = he - hs
                xx_b, zz_b, xs_b, y_b, bb_t, accs = bstate[b]
                hooks = dict(hooks or {})
                xh = xx_b[:][DK * hh : DK * hh + DK, :]
                zh = zz_b[:][DK * hh : DK * hh + DK, :]
                ch_t = ps.tile([128, 455], F32, tag="ch", bufs=2)
                pviews = []
                bursts = []
                for j in range(he // 128):
                    s0 = 128 * j
                    rel = s0 - hs
                    lo = max(0, rel)
                    sp = ps.tile([128, 896], F32, tag="sp", bufs=2)
                    # score matmuls, split at the col-512 psum bank boundary
                    def score_span(cs, ce):
                        while cs < ce:
                            sl = min(512 * (cs // 512) + 512, ce) - cs
                            nc.tensor.matmul(
                                sp[:, cs : cs + sl],
                                lhsT=xh[:, s0 : s0 + 128],
                                rhs=zh[:, hs + cs : hs + cs + sl],
                                start=True,
                                stop=True,
                                skip_group_check=True,
                            )
                            cs += sl
                    if rel < 0:
                        score_span(0, w)
                    else:
                        nc.tensor.matmul(
                            sp[:, rel : rel + 128],
                            lhsT=idn128_t,
                            rhs=maskt_t,
                            start=True,
                            stop=False,
                            skip_group_check=True,
                        )
                        nc.tensor.matmul(
                            sp[:, rel : rel + 128],
                            lhsT=xh[:, s0 : s0 + 128],
                            rhs=zh[:, s0 : s0 + 128],
                            start=False,
                            stop=True,
                            skip_group_check=True,
                        )
                        score_span(rel + 128, w)
                    bcol = NT_ * hh + j
                    if sched[(pair, u, j)] == "act":
                        p = sb.tile([128, 896], BF, tag="pa", bufs=PBUFS)
                        nc.scalar.activation(
                            p[:, lo:w], sp[:, lo:w], AF.Exp,
                            bias=bb_t[:, bcol : bcol + 1],
                        )
                        pview = p[:]
                    else:
                        p = sb.tile([128, 896], I16, tag="pi", bufs=PBUFS)
                        nc.vector.tensor_scalar(
                            p[:, lo:w], sp[:, lo:w],
                            SCH_A,
                            bb_t[:, 2 * NT_ + bcol : 2 * NT_ + bcol + 1],
                            A.mult, A.add,
                        )
                        pview = p[:].bitcast(BF)

                    if dbg_p is not None and pair == 0:
                        pf = sb.tile([128, 896], F32, tag="pf", bufs=2)
                        nc.vector.tensor_copy(pf[:, lo:w], pview[:, lo:w])
                        nc.sync.dma_start(out=dbg_p[j, :, lo:w], in_=pf[:, lo:w])
                    pviews.append(pview)

                    # PSUM allows one open accumulation group per bank, so a
                    # chunk's PV contributions are emitted as one contiguous
                    # open->close burst once its diagonal P tile exists;
                    # deferred by one j so PE isn't gated on exp latency.
                    def _burst(c=j - hs // 128):
                        v3 = pstate[pair][0]
                        for jj in range(hs // 128 + c + 1):
                            nc.tensor.matmul(
                                ch_t[:, 65 * c : 65 * c + 65],
                                lhsT=pviews[jj][:, 128 * c : 128 * c + 128],
                                rhs=v3[:, jj, :],
                                start=(jj == 0),
                                stop=(jj == hs // 128 + c),
                                skip_group_check=True,
                            )

                    if rel >= 0:
                        bursts.append(_burst)
                    if len(bursts) > PV_LAG:
                        bursts.pop(0)()
                    if j in hooks:
                        hooks.pop(j)()
                while bursts:
                    bursts.pop(0)()
                for hk in hooks.values():  # unit shorter than hook points
                    hk()

                def _epilogue():
                    t0 = hs // 128
                    if dbg_p is not None and pair == 0:
                        chf = sb.tile([128, 455], F32, tag="chf", bufs=2)
                        nc.vector.tensor_copy(chf[:, 0 : 65 * nch], ch_t[:, 0 : 65 * nch])
                        nc.sync.dma_start(out=dbg_ch[:, 0 : 65 * nch], in_=chf[:, 0 : 65 * nch])
                    acc_h = pstate[pair][1]
                    r7 = sb.tile([128, 7], F32, tag="r7", bufs=3)
                    nc.vector.reciprocal(
                        r7[:, 0:nch],
                        ch_t[:].rearrange("p (c w) -> p c w", w=65)[:, 0:nch, 64:65],
                    )
                    for c in range(nch):
                        i = t0 + c
                        nc.vector.scalar_tensor_tensor(
                            y_b[:, 128 * i + DK * hh : 128 * i + DK * hh + DK],
                            ch_t[:, 65 * c : 65 * c + 64],
                            r7[:, c : c + 1],
                            xs_b[:, 128 * i + DK * hh : 128 * i + DK * hh + DK],
                            A.mult,
                            A.add,
                            accum_out=acc_h[:, i : i + 1],
                        )
                    if hh == HPC - 1:
                        _emit_stats(b, u, y_b, accs)

                return _epilogue

            emit_proj(0)
            emit_proj_v(0)
            pending = []

            def pop_epi():
                if len(pending) > EPI_LAG:
                    pending.pop(0)()

            for pair in range(NP):
                def drain_ln(pair=pair):
                    if pending_ln and (len(pending_ln) >= 2 or pair >= NP - 2):
                        pending_ln.pop(0)()
                for k, u in enumerate(porder_):
                    hooks = {2: pop_epi, 8: drain_ln, 12: drain_ln}
                    if k == 0 and pair + 1 < NP:
                        hooks[3] = (lambda pr=pair + 1: emit_proj(pr))
                        hooks[6] = (lambda pr=pair + 1: emit_proj_v(pr))
                    epi = emit_junit(pair, u, hooks=hooks)
                    pending.append(epi)
            for e in pending:
                e()
            while pending_ln:
                pending_ln.pop(0)()

    # Restrict Exp/Ln/Copy to the shared natural_log_exp_and_others table set
    # so the whole kernel uses one ACT table load.
    import concourse.bacc as _bacc_mod

    _orig_tables = _bacc_mod.get_activation_tables

    def _filtered_tables(arch):
        outm = {}
        for name, fns in _orig_tables(arch).items():
            if name != "natural_log_exp_and_others":
                fns = set(fns) - {AF.Exp, AF.Ln, AF.Copy, AF.Square}
            outm[name] = fns
        return outm

    _bacc_mod.get_activation_tables = _filtered_tables
    try:
        nc.compile()
    finally:
        _bacc_mod.get_activation_tables = _orig_tables
    return nc


_GRAPH_CACHE = {}


def _get_graph(apply_affine: bool) -> bass.Bass:
    if apply_affine not in _GRAPH_CACHE:
        _GRAPH_CACHE[apply_affine] = _build_graph(apply_affine)
    return _GRAPH_CACHE[apply_affine]


def _prep_in_maps(x, Wq, bq, Wk, bk, Wv, bv, gamma, beta, apply_affine):
    scale = 1.0 / np.sqrt(np.float64(DK))
    in_maps = []
    for i in range(NCORES):
        dsl = slice(DC * i, DC * (i + 1))
        hsl = slice(HPC * i, HPC * (i + 1))
        x_sl = x[:, :, dsl]                       # [B, S, 128]
        xx_np = x_sl.transpose(0, 2, 1)           # [B, 128, S]
        Wq_h = Wq[hsl].astype(np.float64)
        bq_h = bq[hsl].astype(np.float64)
        Wk_h = Wk[hsl].astype(np.float64)
        # A_h = Wk Wq^T * scale ; z = A x_s ; score += (Wk bq * scale) . x_t
        A_h = np.einsum("hde,hfe->hdf", Wk_h, Wq_h) * scale   # [h, dK, dQ]
        u_h = np.einsum("hde,he->hd", Wk_h, bq_h) * scale     # [h, dK]
        wz = np.zeros((DC, DC), np.float64)
        for hh in range(HPC):
            blk = slice(DK * hh, DK * hh + DK)
            wz[blk, blk] = A_h[hh].T
        wv = np.zeros((DC, DK), np.float64)
        for hh in range(HPC):
            wv[DK * hh : DK * hh + DK, :] = Wv[hsl][hh]
        # per-key bias bb[b, hh, t] = u_h . x_h[:, t]
        bb = np.einsum("hd,bthd->bht", u_h,
                       x_sl.reshape(x.shape[0], x.shape[1], HPC, DK).astype(np.float64))
        bbq_np = bb.reshape(x.shape[0], HPC, S // 128, 128).transpose(0, 1, 3, 2)
        bbs_np = bbq_np * SCH_A + SCH_B
        bbp_np = np.stack([bbq_np, bbs_np], axis=1)  # [B, 2, HPC, 128, NT]
        xs_np = x_sl + bv[hsl].reshape(1, 1, DC)
        m = {
            "xx": np.ascontiguousarray(xx_np).astype(BF_NP),
            "xs": np.ascontiguousarray(xs_np).astype(np.float32),
            "wpack": np.ascontiguousarray(
                np.concatenate([wz, wv], axis=1)
            ).astype(BF_NP),
            "bbp": np.ascontiguousarray(bbp_np).astype(np.float32),
        }
        if apply_affine:
            m["gam"] = np.ascontiguousarray(
                np.tile(gamma[dsl][None, :], (128, 1))
            ).astype(np.float32)
            m["bet"] = np.ascontiguousarray(
                np.tile(beta[dsl][None, :], (128, 1))
            ).astype(np.float32)
        in_maps.append(m)
    return in_maps


def kernel(x, Wq, bq, Wk, bk, Wv, bv, gamma, beta):
    global LAST_RESULTS
    x = np.asarray(x, np.float32)
    Wq = np.asarray(Wq, np.float32)
    bq = np.asarray(bq, np.float32)
    Wk = np.asarray(Wk, np.float32)
    bk = np.asarray(bk, np.float32)
    Wv = np.asarray(Wv, np.float32)
    bv = np.asarray(bv, np.float32)
    gamma = np.asarray(gamma, np.float32)
    beta = np.asarray(beta, np.float32)

    apply_affine = not (
        np.allclose(gamma, 1.0, atol=0.0, rtol=0.0)
        and np.allclose(beta, 0.0, atol=0.0, rtol=0.0)
    )
    fake_ar = bool(int(os.environ.get("KERNEL_FAKE_AR", "0")))
    nc = _get_graph(apply_affine) if not fake_ar else _build_graph(apply_affine, fake_ar=True)

    in_maps = _prep_in_maps(x, Wq, bq, Wk, bk, Wv, bv, gamma, beta, apply_affine)

    res = run_bass_kernel_spmd(
        nc,
        in_maps,
        core_ids=list(range(NCORES)),
        trace=bool(int(os.environ.get("KERNEL_TRACE", "0"))),
    )
    LAST_RESULTS = res
    outs = [np.asarray(r["out"], np.float32) for r in res.results]
    return np.concatenate(outs, axis=2)


if __name__ == "__main__":
    nc = _build_graph(False)
    print("graph built ok:", len(nc.inst_map), "instructions")


# revision 21
# speedup vs baseline: 1.2520x; 1.1900x over previous
"""Trainium2 Bass kernel: per-head attention + residual + LayerNorm.

Problem shape: x [4, 2048, 1024], 16 heads of dk=64, causal softmax attention
with per-head Q/K/V linear projections, residual add, LayerNorm(D).

Sharding (8 cores): head-parallel. Core i owns heads (2i, 2i+1) = feature
columns 128*i : 128*(i+1). Only cross-core traffic: per-(batch,unit) AllReduce
of LayerNorm partial sums.

v2 design (vs the 202us baseline, whose bottleneck was the ACT engine at 77%
busy running exp over the causal score area):
- Softmax-invariance scores: sp[t,s] = x_t^T (Wk Wq^T/sqrt(dk)) x_s + u.x_t
  (query-only bias terms cancel in softmax). The per-key bias u.x_t is
  computed on the HOST and folded into the exp instruction's per-partition
  bias operand - no bias-row augmentation, no on-device bias adds.
- zz = block-diag(A_h0^T, A_h1^T) @ xx projects BOTH heads in one matmul
  stream ([128,S]); per-head scores contract over a 64-partition slice.
- exp is split across the ACT engine (true Exp) and DVE (Schraudolph:
  bf16 = bitcast(int16(184.665*s + bias)), a single tensor_scalar with
  ~1.8% sigma error that mostly cancels in softmax). A build-time greedy
  balancer also assigns the zz/v PSUM->SBUF copies to ACT (AF.Copy, same
  act table) or DVE. GPSIMD cannot touch PSUM, so Pool instead absorbs the
  SBUF-only LayerNorm/stats elementwise work.
- PV reoriented to P^T V: out[s-chunk 128, 65] with lhsT = P chunk; free
  size 65 per matmul (vs 512) halves PE PV time, writes y directly (no PE
  transposes, no O^T drain copies), and the rhs ones-column lands the
  softmax denominator in chunk col 64. V bias is pre-added into xs on host.
- Query-units of (896, 896, 256) cols keep the PV chunk tile (nch x 65 fp32
  <= 455) inside one PSUM bank; processed [u1, u2, u0] so the last
  (batch,unit) stats AllReduce covers the smallest unit (short tail).
- LayerNorm output (emit_ln) is emitted ~one pair after its AllReduce
  fires, spreading Pool/out-DMA work over compute instead of a serial tail.

Self-contained: hardcodes all shapes; no sibling imports.
"""

import os
import numpy as np
import ml_dtypes

import concourse.bass as bass
import concourse.bacc as bacc
import concourse.mybir as mybir
from concourse.tile import TileContext
from concourse.bass_utils import run_bass_kernel_spmd

B, S, D, H = 4, 2048, 1024, 16
NCORES = 8
HPC = H // NCORES          # heads per core = 2
DK = D // H                # 64
DC = HPC * DK              # 128 feature cols per core
NT = S // 128              # 16 row tiles of 128
EPS = 1e-5
MASKNEG = -40.0
EPI_LAG = 1
PV_LAG = int(os.environ.get("K_PV_LAG", "3"))
PBUFS = 18
BF = mybir.dt.bfloat16
F32 = mybir.dt.float32
I16 = mybir.dt.int16
BF_NP = ml_dtypes.bfloat16
RG = [list(range(NCORES))]
A = mybir.AluOpType
AF = mybir.ActivationFunctionType

# Schraudolph exp constants (bf16 = bitcast(int16(SCH_A * v + SCH_B)))
SCH_A = 184.6650292  # 2^7 / ln 2
SCH_B = 16256.0 - 7.32  # 127 * 2^7, centered (hw rounds to nearest)

# Query units (hs, he, nch): 512-wide so the score tile is one PSUM bank
# (4-deep sp ring hides exp latency). Order keeps a long unit first (proj
# hooks) and last (tail LN drain room).
UNITS = [(0, 512, 4), (512, 1024, 4), (1024, 1536, 4), (1536, 2048, 4)]
PORDER = [1, 2, 0, 3]
NU = len(UNITS)


def _units_for(S_):
    if S_ == 2048:
        return UNITS, PORDER
    nch = S_ // 128
    assert nch <= 7
    return [(0, S_, nch)], [0]

# greedy ACT/DVE balance cost model (ns): per-instr, per-col
ACT_COST = (190.0, 0.833)
DVE_COST = (127.0, 1.042)
# recurring per-pair engine loads not part of the flexible item list
DVE_PAIR_FIXED = float(os.environ.get("K_DVE_PAIR", "3700"))
ACT_PAIR_FIXED = float(os.environ.get("K_ACT_PAIR", "1250"))

LAST_RESULTS = None  # BassKernelResults of the last run (for test harness)


def _exp_schedule(B_=B, S_=S):
    """Greedy ACT/DVE balance over exp tiles and psum->sbuf copies in
    emission order. Returns {key: 'act'|'dve'}."""
    units, porder = _units_for(S_)
    NT_ = S_ // 128
    load = {"act": 0.0, "dve": 0.0}
    sched = {}

    def assign(key, n):
        cost_a = ACT_COST[0] + ACT_COST[1] * n
        cost_d = DVE_COST[0] + DVE_COST[1] * n
        if load["act"] + cost_a <= load["dve"] + cost_d:
            sched[key] = "act"
            load["act"] += cost_a
        else:
            sched[key] = "dve"
            load["dve"] += cost_d

    for pair in range(B_ * HPC):
        b, hh = divmod(pair, HPC)
        load["dve"] += DVE_PAIR_FIXED
        load["act"] += ACT_PAIR_FIXED
        if hh == 0:
            for c in range(S_ // 512):
                assign(("z", b, c), 512)
        for g in range((NT_ + 7) // 8):
            assign(("v", pair, g), 64 * min(8, NT_ - 8 * g))
        for u in porder:
            hs, he, nch = units[u]
            w = he - hs
            for j in range(he // 128):
                lo = max(0, 128 * j - hs)
                assign((pair, u, j), w - lo)
            if hh == 1:
                for i in range(nch):
                    # sum-of-squares: ACT Square+accum vs DVE stt
                    cost_a = 190.0 + 187.0 + 128 * ACT_COST[1]
                    cost_d = DVE_COST[0] + 128 * DVE_COST[1]
                    if load["act"] + cost_a <= load["dve"] + cost_d:
                        sched[("sq", b, u, i)] = "act"
                        load["act"] += cost_a
                    else:
                        sched[("sq", b, u, i)] = "dve"
                        load["dve"] += cost_d
    return sched


def _build_graph(apply_affine: bool, B_: int = B, S_: int = S, rg=None, fake_ar: bool = False) -> bass.Bass:
    nc = bacc.Bacc()
    NT_ = S_ // 128
    if rg is None:
        rg = RG
    sched = _exp_schedule(B_, S_)
    units_, porder_ = _units_for(S_)
    NU_ = len(units_)

    xx = nc.declare_dram_parameter("xx", [B_, DC, S_], BF, isOutput=False)
    xs = nc.declare_dram_parameter("xs", [B_, S_, DC], F32, isOutput=False)
    wpack = nc.declare_dram_parameter("wpack", [DC, DC + DK], BF, isOutput=False)
    # bbp[b, 0] = plain per-key bias (ACT exp bias); bbp[b, 1] = Schraudolph
    bbp = nc.declare_dram_parameter("bbp", [B_, 2, HPC, 128, NT_], F32, isOutput=False)
    if apply_affine:
        gam = nc.declare_dram_parameter("gam", [128, DC], F32, isOutput=False)
        bet = nc.declare_dram_parameter("bet", [128, DC], F32, isOutput=False)
    out = nc.declare_dram_parameter("out", [B_, S_, DC], F32, isOutput=True)
    dbg_y = None
    if os.environ.get("K_DBG_Y"):
        dbg_y = nc.declare_dram_parameter("dbg_y", [B_, 128, S_], F32, isOutput=True)
    dbg_p = None
    if os.environ.get("K_DBG_P"):
        dbg_p = nc.declare_dram_parameter("dbg_p", [NT_, 128, 512], F32, isOutput=True)
        dbg_ch = nc.declare_dram_parameter("dbg_ch", [128, 260], F32, isOutput=True)

    # constants baked into the NEFF: [idn128 | upper-triangle MASKNEG] bf16
    trineg_np = np.where(
        np.arange(128)[:, None] > np.arange(128)[None, :], MASKNEG, 0.0
    ).astype(np.float32)
    imask_h = nc.inline_tensor(
        np.concatenate([np.eye(128, dtype=np.float32), trineg_np], axis=1).astype(BF_NP),
        name="imask",
    )

    # collective bounce buffers: LayerNorm stats per (batch, unit):
    # [2(sum,sumsq), 128 rows, chunks]
    stats_in = nc.dram_tensor("stats_in", [B_, NU_, 2, 128, 7], F32)
    stats_out = nc.dram_tensor("stats_out", [B_, NU_, 2, 128, 7], F32, addr_space="Shared")

    NP = B_ * HPC

    def copy_ps(key, dst, src):
        if sched[key] == "act":
            nc.scalar.activation(dst, src, AF.Copy)
        else:
            nc.vector.tensor_copy(dst, src)

    with TileContext(nc) as tc:
        with (
            tc.tile_pool(name="consts", bufs=1) as cpool,
            tc.tile_pool(name="sb", bufs=2) as sb,
            tc.tile_pool(name="ps", bufs=1, space="PSUM") as ps,
        ):
            # ---- constants (first-needed first) ----
            wp_t = cpool.tile([DC, DC + DK], BF, tag="wp")
            nc.sync.dma_start(out=wp_t[:], in_=wpack[:, :])
            wz_t = wp_t[:][:, 0:DC]
            wv_t = wp_t[:][:, DC : DC + DK]
            imaskq_t = cpool.tile([128, 256], BF, tag="imask")
            nc.gpsimd.dma_start(out=imaskq_t[:], in_=imask_h[:, :])
            idn128_t = imaskq_t[:][:, 0:128]
            maskt_t = imaskq_t[:][:, 128:256]
            eps_t = cpool.tile([128, 1], F32, tag="eps")
            nc.vector.memset(eps_t[:], EPS)
            if apply_affine:
                gam_t = cpool.tile([128, DC], F32, tag="gam")
                nc.gpsimd.dma_start(out=gam_t[:], in_=gam[:, :])
                bet_t = cpool.tile([128, DC], F32, tag="bet")
                nc.gpsimd.dma_start(out=bet_t[:], in_=bet[:, :])

            y_tiles = {}
            bstate = {}
            pstate = {}
            pending_ln = []

            def _emit_stats(b, u, y_b, accs):
                hs, he, nch = units_[u]
                t0 = hs // 128
                pk = sb.tile([128, 14], F32, tag="pk", bufs=3)
                nc.gpsimd.tensor_add(
                    pk[:, 0:nch], accs[0][:, t0 : t0 + nch], accs[1][:, t0 : t0 + nch]
                )
                for i in range(t0, t0 + nch):
                    scr = sb.tile([128, 128], F32, tag="scr")
                    if sched[("sq", b, u, i - t0)] == "act":
                        nc.scalar.activation(
                            scr[:],
                            y_b[:, 128 * i : 128 * i + 128],
                            AF.Square,
                            accum_out=pk[:, nch + i - t0 : nch + 1 + i - t0],
                        )
                    else:
                        nc.vector.scalar_tensor_tensor(
                            scr[:],
                            y_b[:, 128 * i : 128 * i + 128],
                            1.0,
                            y_b[:, 128 * i : 128 * i + 128],
                            A.mult,
                            A.mult,
                            accum_out=pk[:, nch + i - t0 : nch + 1 + i - t0],
                        )
                nc.sync.dma_start(
                    out=stats_in[b, u, :, :, 0:nch].rearrange("c p t -> p c t"),
                    in_=pk[:, 0 : 2 * nch].rearrange("p (c t) -> p c t", t=nch),
                )
                if dbg_y is not None:
                    nc.sync.dma_start(
                        out=dbg_y[b, :, hs:he], in_=y_b[:, hs:he]
                    )
                if fake_ar:
                    nc.sync.dma_start(
                        out=stats_out[b, u, :, :, 0:nch], in_=stats_in[b, u, :, :, 0:nch]
                    )
                else:
                    nc.gpsimd.collective_compute(
                        "AllReduce",
                        A.add,
                        replica_groups=rg,
                        ins=[stats_in[b, u].opt()],
                        outs=[stats_out[b, u].opt()],
                    )
                pending_ln.append(lambda b=b, u=u: emit_ln(b, u, y_b))

            def emit_ln(b, u, y_b):
                hs, he, nch = units_[u]
                t0 = hs // 128
                red = sb.tile([128, 14], F32, tag="red", bufs=3)
                nc.sync.dma_start(
                    out=red[:, 0 : 2 * nch].rearrange("p (c t) -> p c t", t=nch),
                    in_=stats_out[b, u, :, :, 0:nch].rearrange("c p t -> p c t"),
                )
                mean = sb.tile([128, 7], F32, tag="mean", bufs=3)
                nc.gpsimd.tensor_scalar(mean[:, 0:nch], red[:, 0:nch], 1.0 / D, None, A.mult)
                msq = sb.tile([128, 7], F32, tag="msq", bufs=3)
                nc.gpsimd.tensor_mul(msq[:, 0:nch], mean[:, 0:nch], mean[:, 0:nch])
                var = sb.tile([128, 7], F32, tag="var", bufs=3)
                nc.gpsimd.tensor_scalar(
                    var[:, 0:nch], red[:, nch : 2 * nch], 1.0 / D, None, A.mult
                )
                nc.gpsimd.tensor_tensor(
                    var[:, 0:nch], var[:, 0:nch], msq[:, 0:nch], A.subtract
                )
                lnv = sb.tile([128, 7], F32, tag="lnv", bufs=3)
                nc.scalar.activation(lnv[:, 0:nch], var[:, 0:nch], AF.Ln, bias=eps_t[:])
                rstd = sb.tile([128, 7], F32, tag="rstd", bufs=3)
                nc.scalar.activation(rstd[:, 0:nch], lnv[:, 0:nch], AF.Exp, scale=-0.5)
                ostb = sb.tile([128, 128 * 7], F32, tag="ost", bufs=2)
                for k in range(nch):
                    i = t0 + k
                    nc.gpsimd.tensor_scalar(
                        ostb[:, 128 * k : 128 * k + 128],
                        y_b[:, 128 * i : 128 * i + 128],
                        mean[:, k : k + 1],
                        rstd[:, k : k + 1],
                        A.subtract,
                        A.mult,
                    )
                    if apply_affine:
                        nc.gpsimd.tensor_mul(
                            ostb[:, 128 * k : 128 * k + 128],
                            ostb[:, 128 * k : 128 * k + 128],
                            gam_t[:],
                        )
                        nc.gpsimd.tensor_add(
                            ostb[:, 128 * k : 128 * k + 128],
                            ostb[:, 128 * k : 128 * k + 128],
                            bet_t[:],
                        )
                nc.sync.dma_start(
                    out=out[b, 128 * t0 : 128 * (t0 + nch), :].rearrange(
                        "(i p) d -> p i d", p=128
                    ),
                    in_=ostb[:, 0 : 128 * nch].rearrange("p (i d) -> p i d", d=128),
                )

            def emit_proj(pair):
                b, hh = divmod(pair, HPC)
                if hh == 0:
                    xx_b = sb.tile([128, S_], BF, tag="xx", name=f"xx{b}", bufs=2)
                    nc.sync.dma_start(out=xx_b[:, 0 : S_ // 2], in_=xx[b, :, 0 : S_ // 2])
                    nc.sync.dma_start(out=xx_b[:, S_ // 2 : S_], in_=xx[b, :, S_ // 2 : S_])
                    bb_t = sb.tile([128, 2 * HPC * NT_], F32, tag="bb", name=f"bb{b}", bufs=2)
                    nc.sync.dma_start(
                        out=bb_t[:].rearrange("p (v h j) -> p v h j", v=2, h=HPC),
                        in_=bbp[b].rearrange("v h p j -> p v h j"),
                    )
                    # zz projection: both heads at once via block-diag wz
                    zz_b = sb.tile([128, S_], BF, tag="zz", name=f"zz{b}", bufs=2)
                    for c in range(S_ // 512):
                        zp = ps.tile([128, 512], F32, tag="op", bufs=2, name=f"zp{b}_{c}")
                        nc.tensor.matmul(
                            zp[:],
                            lhsT=wz_t,
                            rhs=xx_b[:, 512 * c : 512 * c + 512],
                            start=True,
                            stop=True,
                        )
                        copy_ps(("z", b, c), zz_b[:, 512 * c : 512 * c + 512], zp[:])
                    xs_b = sb.tile([128, S_], F32, tag="xs", name=f"xs{b}")
                    nc.sync.dma_start(
                        out=xs_b[:].rearrange("p (i d) -> p i d", d=128),
                        in_=xs[b].rearrange("(i p) d -> p i d", p=128),
                    )
                    y_b = sb.tile([128, S_], F32, tag="y", name=f"y{b}", bufs=3)
                    y_tiles[b] = y_b
                    bstate[b] = (xx_b, zz_b, xs_b, y_b, bb_t, {})
                xx_b, zz_b, xs_b, y_b, bb_t, accs = bstate[b]
                acc_h = sb.tile([128, NT_], F32, tag=f"acc{hh}", name=f"acc{pair}", bufs=2)
                accs[hh] = acc_h
                pstate[pair] = [None, acc_h]

            def emit_proj_v(pair):
                b, hh = divmod(pair, HPC)
                xx_b, zz_b, xs_b, y_b, bb_t, accs = bstate[b]
                # V projection for this head: v[t, 65j:65j+64], ones at col 64
                v_t = sb.tile([128, NT_ * 65], BF, tag="v", name=f"v{pair}", bufs=3)
                v3 = v_t[:].rearrange("p (t w) -> p t w", w=65)
                nc.gpsimd.memset(v3[:, :, 64:65], 1.0)
                for g in range((NT_ + 7) // 8):
                    gn = min(8, NT_ - 8 * g)
                    vp = ps.tile([128, 512], F32, tag="op", bufs=2, name=f"vp{pair}_{g}")
                    for uu in range(gn):
                        j = 8 * g + uu
                        nc.tensor.matmul(
                            vp[:, DK * uu : DK * uu + DK],
                            lhsT=xx_b[:][DK * hh : DK * hh + DK, 128 * j : 128 * j + 128],
                            rhs=wv_t[DK * hh : DK * hh + DK, :],
                            start=True,
                            stop=True,
                        )
                    copy_ps(
                        ("v", pair, g),
                        v3[:, 8 * g : 8 * g + gn, 0:64],
                        vp[:, 0 : DK * gn].rearrange("q (t w) -> q t w", w=DK),
                    )
                pstate[pair][0] = v3

            def emit_junit(pair, u, hooks=None):
                """Score/exp/PV loop for one query unit; returns the deferred
                normalize epilogue closure."""
                b, hh = divmod(pair, HPC)
                hs, he, nch = units_[u]
                w = he - hs
                xx_b, zz_b, xs_b, y_b, bb_t, accs = bstate[b]
                hooks = dict(hooks or {})
                xh = xx_b[:][DK * hh : DK * hh + DK, :]
                zh = zz_b[:][DK * hh : DK * hh + DK, :]
                ch_t = ps.tile([128, 260], F32, tag="ch", bufs=2)
                pviews = []
                bursts = []
                for j in range(he // 128):
                    s0 = 128 * j
                    rel = s0 - hs
                    lo = max(0, rel)
                    sp = ps.tile([128, 512], F32, tag="sp", bufs=4)
                    # score matmuls, split at the col-512 psum bank boundary
                    def score_span(cs, ce):
                        while cs < ce:
                            sl = min(512 * (cs // 512) + 512, ce) - cs
                            nc.tensor.matmul(
                                sp[:, cs : cs + sl],
                                lhsT=xh[:, s0 : s0 + 128],
                                rhs=zh[:, hs + cs : hs + cs + sl],
                                start=True,
                                stop=True,
                                skip_group_check=True,
                            )
                            cs += sl
                    if rel < 0:
                        score_span(0, w)
                    else:
                        nc.tensor.matmul(
                            sp[:, rel : rel + 128],
                            lhsT=idn128_t,
                            rhs=maskt_t,
                            start=True,
                            stop=False,
                            skip_group_check=True,
                        )
                        nc.tensor.matmul(
                            sp[:, rel : rel + 128],
                            lhsT=xh[:, s0 : s0 + 128],
                            rhs=zh[:, s0 : s0 + 128],
                            start=False,
                            stop=True,
                            skip_group_check=True,
                        )
                        score_span(rel + 128, w)
                    bcol = NT_ * hh + j
                    if sched[(pair, u, j)] == "act":
                        p = sb.tile([128, 512], BF, tag="pa", bufs=PBUFS)
                        nc.scalar.activation(
                            p[:, lo:w], sp[:, lo:w], AF.Exp,
                            bias=bb_t[:, bcol : bcol + 1],
                        )
                        pview = p[:]
                    else:
                        p = sb.tile([128, 512], I16, tag="pi", bufs=PBUFS)
                        nc.vector.tensor_scalar(
                            p[:, lo:w], sp[:, lo:w],
                            SCH_A,
                            bb_t[:, 2 * NT_ + bcol : 2 * NT_ + bcol + 1],
                            A.mult, A.add,
                        )
                        pview = p[:].bitcast(BF)

                    if dbg_p is not None and pair == 0:
                        pf = sb.tile([128, 512], F32, tag="pf", bufs=2)
                        nc.vector.tensor_copy(pf[:, lo:w], pview[:, lo:w])
                        nc.sync.dma_start(out=dbg_p[j, :, lo:w], in_=pf[:, lo:w])
                    pviews.append(pview)

                    # PSUM allows one open accumulation group per bank, so a
                    # chunk's PV contributions are emitted as one contiguous
                    # open->close burst once its diagonal P tile exists;
                    # deferred by one j so PE isn't gated on exp latency.
                    def _burst(c=j - hs // 128):
                        v3 = pstate[pair][0]
                        for jj in range(hs // 128 + c + 1):
                            nc.tensor.matmul(
                                ch_t[:, 65 * c : 65 * c + 65],
                                lhsT=pviews[jj][:, 128 * c : 128 * c + 128],
                                rhs=v3[:, jj, :],
                                start=(jj == 0),
                                stop=(jj == hs // 128 + c),
                                skip_group_check=True,
                            )

                    if rel >= 0:
                        bursts.append(_burst)
                    if len(bursts) > PV_LAG:
                        bursts.pop(0)()
                    if j in hooks:
                        hooks.pop(j)()
                while bursts:
                    bursts.pop(0)()
                for hk in hooks.values():  # unit shorter than hook points
                    hk()

                def _epilogue():
                    t0 = hs // 128
                    if dbg_p is not None and pair == 0:
                        chf = sb.tile([128, 260], F32, tag="chf", bufs=2)
                        nc.vector.tensor_copy(chf[:, 0 : 65 * nch], ch_t[:, 0 : 65 * nch])
                        nc.sync.dma_start(out=dbg_ch[:, 0 : 65 * nch], in_=chf[:, 0 : 65 * nch])
                    acc_h = pstate[pair][1]
                    r7 = sb.tile([128, 7], F32, tag="r7", bufs=3)
                    nc.vector.reciprocal(
                        r7[:, 0:nch],
                        ch_t[:].rearrange("p (c w) -> p c w", w=65)[:, 0:nch, 64:65],
                    )
                    for c in range(nch):
                        i = t0 + c
                        nc.vector.scalar_tensor_tensor(
                            y_b[:, 128 * i + DK * hh : 128 * i + DK * hh + DK],
                            ch_t[:, 65 * c : 65 * c + 64],
                            r7[:, c : c + 1],
                            xs_b[:, 128 * i + DK * hh : 128 * i + DK * hh + DK],
                            A.mult,
                            A.add,
                            accum_out=acc_h[:, i : i + 1],
                        )
                    if hh == HPC - 1:
                        _emit_stats(b, u, y_b, accs)

                return _epilogue

            emit_proj(0)
            emit_proj_v(0)
            pending = []

            def pop_epi():
                if len(pending) > EPI_LAG:
                    pending.pop(0)()

            for pair in range(NP):
                def drain_ln(pair=pair):
                    if pending_ln and (len(pending_ln) >= 2 or pair >= NP - 2):
                        pending_ln.pop(0)()
                for k, u in enumerate(porder_):
                    hooks = {2: pop_epi, 8: drain_ln, 12: drain_ln}
                    if k == 0 and pair + 1 < NP:
                        hooks[3] = (lambda pr=pair + 1: emit_proj(pr))
                        hooks[6] = (lambda pr=pair + 1: emit_proj_v(pr))
                    epi = emit_junit(pair, u, hooks=hooks)
                    pending.append(epi)
            for e in pending:
                e()
            while pending_ln:
                pending_ln.pop(0)()

    # Restrict Exp/Ln/Copy to the shared natural_log_exp_and_others table set
    # so the whole kernel uses one ACT table load.
    import concourse.bacc as _bacc_mod

    _orig_tables = _bacc_mod.get_activation_tables

    def _filtered_tables(arch):
        outm = {}
        for name, fns in _orig_tables(arch).items():
            if name != "natural_log_exp_and_others":
                fns = set(fns) - {AF.Exp, AF.Ln, AF.Copy, AF.Square}
            outm[name] = fns
        return outm

    _bacc_mod.get_activation_tables = _filtered_tables
    try:
        nc.compile()
    finally:
        _bacc_mod.get_activation_tables = _orig_tables
    return nc


_GRAPH_CACHE = {}


def _get_graph(apply_affine: bool) -> bass.Bass:
    if apply_affine not in _GRAPH_CACHE:
        _GRAPH_CACHE[apply_affine] = _build_graph(apply_affine)
    return _GRAPH_CACHE[apply_affine]


def _prep_in_maps(x, Wq, bq, Wk, bk, Wv, bv, gamma, beta, apply_affine):
    scale = 1.0 / np.sqrt(np.float64(DK))
    in_maps = []
    for i in range(NCORES):
        dsl = slice(DC * i, DC * (i + 1))
        hsl = slice(HPC * i, HPC * (i + 1))
        x_sl = x[:, :, dsl]                       # [B, S, 128]
        xx_np = x_sl.transpose(0, 2, 1)           # [B, 128, S]
        Wq_h = Wq[hsl].astype(np.float64)
        bq_h = bq[hsl].astype(np.float64)
        Wk_h = Wk[hsl].astype(np.float64)
        # A_h = Wk Wq^T * scale ; z = A x_s ; score += (Wk bq * scale) . x_t
        A_h = np.einsum("hde,hfe->hdf", Wk_h, Wq_h) * scale   # [h, dK, dQ]
        u_h = np.einsum("hde,he->hd", Wk_h, bq_h) * scale     # [h, dK]
        wz = np.zeros((DC, DC), np.float64)
        for hh in range(HPC):
            blk = slice(DK * hh, DK * hh + DK)
            wz[blk, blk] = A_h[hh].T
        wv = np.zeros((DC, DK), np.float64)
        for hh in range(HPC):
            wv[DK * hh : DK * hh + DK, :] = Wv[hsl][hh]
        # per-key bias bb[b, hh, t] = u_h . x_h[:, t]
        bb = np.einsum("hd,bthd->bht", u_h,
                       x_sl.reshape(x.shape[0], x.shape[1], HPC, DK).astype(np.float64))
        bbq_np = bb.reshape(x.shape[0], HPC, S // 128, 128).transpose(0, 1, 3, 2)
        bbs_np = bbq_np * SCH_A + SCH_B
        bbp_np = np.stack([bbq_np, bbs_np], axis=1)  # [B, 2, HPC, 128, NT]
        xs_np = x_sl + bv[hsl].reshape(1, 1, DC)
        m = {
            "xx": np.ascontiguousarray(xx_np).astype(BF_NP),
            "xs": np.ascontiguousarray(xs_np).astype(np.float32),
            "wpack": np.ascontiguousarray(
                np.concatenate([wz, wv], axis=1)
            ).astype(BF_NP),
            "bbp": np.ascontiguousarray(bbp_np).astype(np.float32),
        }
        if apply_affine:
            m["gam"] = np.ascontiguousarray(
                np.tile(gamma[dsl][None, :], (128, 1))
            ).astype(np.float32)
            m["bet"] = np.ascontiguousarray(
                np.tile(beta[dsl][None, :], (128, 1))
            ).astype(np.float32)
        in_maps.append(m)
    return in_maps


def kernel(x, Wq, bq, Wk, bk, Wv, bv, gamma, beta):
    global LAST_RESULTS
    x = np.asarray(x, np.float32)
    Wq = np.asarray(Wq, np.float32)
    bq = np.asarray(bq, np.float32)
    Wk = np.asarray(Wk, np.float32)
    bk = np.asarray(bk, np.float32)
    Wv = np.asarray(Wv, np.float32)
    bv = np.asarray(bv, np.float32)
    gamma = np.asarray(gamma, np.float32)
    beta = np.asarray(beta, np.float32)

    apply_affine = not (
        np.allclose(gamma, 1.0, atol=0.0, rtol=0.0)
        and np.allclose(beta, 0.0, atol=0.0, rtol=0.0)
    )
    fake_ar = bool(int(os.environ.get("KERNEL_FAKE_AR", "0")))
    nc = _get_graph(apply_affine) if not fake_ar else _build_graph(apply_affine, fake_ar=True)

    in_maps = _prep_in_maps(x, Wq, bq, Wk, bk, Wv, bv, gamma, beta, apply_affine)

    res = run_bass_kernel_spmd(
        nc,
        in_maps,
        core_ids=list(range(NCORES)),
        trace=bool(int(os.environ.get("KERNEL_TRACE", "0"))),
    )
    LAST_RESULTS = res
    outs = [np.asarray(r["out"], np.float32) for r in res.results]
    return np.concatenate(outs, axis=2)


if __name__ == "__main__":
    nc = _build_graph(False)
    print("graph built ok:", len(nc.inst_map), "instructions")
